# revision 21
# baseline (speedup 1.0000x reference)
"""GIN message-passing classifier on 8 Trainium2 NeuronCores.

Strategy (graph/node partition, data parallel):
  - Nodes are split into 8 equal contiguous shards (12500 nodes/core); each
    core owns the edges whose *destination* lands in its shard.
  - Host pre-sorts edges by dst, groups them per 128-node tile, pads each
    tile's edge list to a multiple of 128 ("chunks").  Chunk counts are taken
    as the max over cores so all 8 cores run one identical program (SPMD).
  - On device, chunks of 128 edges are gathered with batched indirect DMA
    (h[src] rows) and scatter-added into the tile's PSUM accumulator with a
    one-hot selector matmul built on the fly by the vector engine (is_equal
    against an iota row).  The GIN self term (2*h) is one extra matmul with a
    constant 2*I selector on contiguously-loaded own rows.
  - BatchNorm batch statistics come from per-core partials (sum(X) and the
    second moment X^T X pushed through W analytically), combined with a tiny
    [2,256] AllReduce; scale/shift are folded into W on device, so the
    per-node epilogue is matmul + fused relu.
  - h_new is exchanged between layers with an AllGather (rows = node shards).
  - Readout (segment-sum per graph) uses the same one-hot-selector matmul
    into per-core local graph slots, a small AllGather, and a host-planned
    slot->graph selector reduce; the graph-level MLP is replicated.
"""

import numpy as np

import concourse.bass as bass
import concourse.mybir as mybir
import concourse.tile as tile
from concourse import bacc
from concourse.bass_utils import run_bass_kernel_spmd

P = 128


def _T(tc, *args, **kw):
    t, _free = tc.tile(*args, **kw)
    return t


F32 = mybir.dt.float32
I32 = mybir.dt.int32
AF = mybir.ActivationFunctionType
OP = mybir.AluOpType
AX = mybir.AxisListType
EPS_BN = 1e-5


class Cfg:
    def __init__(self, N=100000, E=1000000, G=512, IN_FEATS=78, HID=256,
                 MLP0=512, MLP1=256, NCLS=204, NC=8, K_GATHER=8):
        self.N, self.E, self.G = N, E, G
        self.IN_FEATS, self.HID = IN_FEATS, HID
        self.MLP0, self.MLP1, self.NCLS = MLP0, MLP1, NCLS
        self.NC, self.K = NC, K_GATHER
        assert N % NC == 0
        self.SHARD = N // NC
        self.NT = (self.SHARD + P - 1) // P
        self.GT = (G + P - 1) // P
        self.NBLK = 4
        # block b = all cores' quarter-b of their shard; quarters are
        # tile-aligned so pipelined AllGathers unblock gather segments.
        tq = (self.NT + 3) // 4           # tiles per quarter (last smaller)
        self.TQ = [tq, tq, tq, self.NT - 3 * tq]
        qb = [min(t * P, self.SHARD) for t in np.cumsum([0] + self.TQ)]
        self.QOFF = qb[:4]                # row offset of quarter q in shard
        self.QB = [qb[i + 1] - qb[i] for i in range(4)]  # rows per quarter
        self.BSZ = [self.NC * q for q in self.QB]        # rows per block
        self.BLKOFF = [0]
        for b in range(3):
            self.BLKOFF.append(self.BLKOFF[-1] + self.BSZ[b])
        assert max(self.BSZ) < 32768, "dma_gather int16 index range"
        self.DPAD = 128  # layer-0 gather row padded to 128 f32 (512B)


def _plan(src, dst, graph_id, cfg):
    """Host-side edge bucketing. Returns per-core data + shared metadata.

    Edges are bucketed by (dst-tile, src-block); the chunk stream is laid out
    block-major (all tiles' block-0 chunks, then block-1, ...) so each
    dma_gather instruction covers one contiguous same-block run of chunks.
    """
    NC, SHARD, NT = cfg.NC, cfg.SHARD, cfg.NT
    NBLK = cfg.NBLK
    qoff = np.asarray(cfg.QOFF + [SHARD], np.int64)
    qb = np.asarray(cfg.QB, np.int64)
    src = np.asarray(src).astype(np.int64).ravel()
    dst = np.asarray(dst).astype(np.int64).ravel()
    gid = np.asarray(graph_id).astype(np.int64).ravel()

    core = dst // SHARD
    rem = dst % SHARD
    t = rem // P
    loc = rem % P
    score = src // SHARD
    soff = src % SHARD
    b = np.searchsorted(qoff, soff, side="right") - 1
    key = (core * NT + t) * NBLK + b
    order = np.argsort(key, kind="stable")
    key = key[order]
    srcl = (score * qb[b] + (soff - qoff[b]))[order]
    assert srcl.max() < 32768
    loc = loc[order]
    cnt = np.bincount(key, minlength=NC * NT * NBLK).reshape(NC, NT, NBLK)
    c_tb = ((cnt + P - 1) // P).max(axis=0).astype(np.int64)  # [NT, NBLK]
    segcnt = c_tb.sum(axis=0)                                  # [NBLK]
    seg0 = np.zeros(NBLK, np.int64)
    seg0[1:] = np.cumsum(segcnt)[:-1]
    # chunk-stream start of each (t, b) run
    colb0 = np.zeros((NT, NBLK), np.int64)
    for bb in range(NBLK):
        colb0[0, bb] = seg0[bb]
        colb0[1:, bb] = seg0[bb] + np.cumsum(c_tb[:-1, bb])
    C = max(int(c_tb.sum()), 1)

    starts = np.zeros(NC * NT * NBLK + 1, np.int64)
    starts[1:] = np.cumsum(cnt.ravel())
    rank = np.arange(len(key)) - starts[key]
    ch = rank // P
    p = rank % P
    tt = (key // NBLK) % NT
    bb_ = key % NBLK
    cc = key // (NT * NBLK)
    s = colb0[tt, bb_] + ch  # stream chunk index
    g = s * P + p            # global slot position

    idx16 = np.zeros((NC, C * P), np.int16)
    dstloc = np.full((NC, P, C), -1.0, np.float32)
    idx16[cc, g] = srcl.astype(np.int16)
    dstloc[cc, p, s] = loc.astype(np.float32)
    # wrapped layout: position g -> [g % 16, g // 16], replicated on 8 Q7 cores
    idx16w = np.zeros((NC, P, (C * P) // 16), np.int16)
    for c in range(NC):
        w = idx16[c].reshape((C * P) // 16, 16).T  # [16, 8C]
        idx16w[c] = np.tile(w, (8, 1))

    gidloc = np.full((NC, P, NT), -1.0, np.float32)
    growid = np.full((P, NC), -1.0, np.float32)
    for c in range(NC):
        gg = gid[c * SHARD:(c + 1) * SHARD]
        gb, gm = int(gg[0]), int(gg[-1])
        assert gm - gb < P, "graph span exceeds 128 per core"
        arr = np.full(NT * P, -1.0, np.float32)
        arr[:SHARD] = (gg - gb).astype(np.float32)
        gidloc[c] = arr.reshape(NT, P).T
        jj = np.arange(P)
        sel = (gb + jj) <= gm
        growid[sel, c] = (gb + jj[sel]).astype(np.float32)

    return dict(c_tb=c_tb, C=C, colb0=colb0, seg0=seg0, segcnt=segcnt,
                idx16=idx16w, dstloc=dstloc, gidloc=gidloc, growid=growid)


def _build(nc, cfg, meta, coll=True, gathers=True):
    """Trace the full Bass/Tile program (shared by all 8 cores)."""
    NT, C, K = cfg.NT, meta["C"], cfg.K
    c_tb, colb0 = meta["c_tb"], meta["colb0"]
    seg0, segcnt = meta["seg0"], meta["segcnt"]
    NBLK = cfg.NBLK
    BSZ, BLKOFF, QOFF, QB, TQ = cfg.BSZ, cfg.BLKOFF, cfg.QOFF, cfg.QB, cfg.TQ
    HID = cfg.HID
    DIMS = [cfg.IN_FEATS, HID, HID]
    N, SHARD, NC_ = cfg.N, cfg.SHARD, cfg.NC
    GT = cfg.GT
    GQ = GT * P

    # ---------------- DRAM I/O ----------------
    h0p = nc.dram_tensor("h0p", [N, cfg.DPAD], F32, kind="ExternalInput").ap()
    hself0 = nc.dram_tensor("hself0", [SHARD, cfg.IN_FEATS], F32,
                            kind="ExternalInput").ap()
    idx_d = nc.dram_tensor("idx16", [P, (C * P) // 16], mybir.dt.int16,
                           kind="ExternalInput").ap()
    dstloc_d = nc.dram_tensor("dstloc", [P, C], F32, kind="ExternalInput").ap()
    gidloc_d = nc.dram_tensor("gidloc", [P, NT], F32, kind="ExternalInput").ap()
    growid_d = nc.dram_tensor("growid", [P, NC_], F32, kind="ExternalInput").ap()
    iota_d = nc.dram_tensor("iota", [P, P], F32, kind="ExternalInput").ap()
    twoI_d = nc.dram_tensor("twoI", [P, P], F32, kind="ExternalInput").ap()
    ident_d = nc.dram_tensor("ident", [P, P], F32, kind="ExternalInput").ap()
    ones_d = nc.dram_tensor("ones", [P, P], F32, kind="ExternalInput").ap()
    W_d = [nc.dram_tensor(f"W{i}", [DIMS[i], HID], F32, kind="ExternalInput").ap()
           for i in range(3)]
    bnp_d = nc.dram_tensor("bnp", [1, 6 * HID], F32,
                           kind="ExternalInput").ap()
    fc1_d = nc.dram_tensor("fc1W", [HID, cfg.MLP0], F32, kind="ExternalInput").ap()
    bn1g_d = nc.dram_tensor("bn1gT", [P, cfg.MLP0 // P], F32,
                            kind="ExternalInput").ap()
    bn1b_d = nc.dram_tensor("bn1bT", [P, cfg.MLP0 // P], F32,
                            kind="ExternalInput").ap()
    mlpW_d = nc.dram_tensor("mlpW", [cfg.MLP0, cfg.MLP1], F32,
                            kind="ExternalInput").ap()
    mbng_d = nc.dram_tensor("mbngT", [P, cfg.MLP1 // P], F32,
                            kind="ExternalInput").ap()
    mbnb_d = nc.dram_tensor("mbnbT", [P, cfg.MLP1 // P], F32,
                            kind="ExternalInput").ap()
    fc2_d = nc.dram_tensor("fc2Wr", [cfg.MLP1, cfg.NCLS], F32,
                           kind="ExternalInput").ap()
    fc2b_d = nc.dram_tensor("fc2br", [1, cfg.NCLS], F32, kind="ExternalInput").ap()
    out_d = nc.dram_tensor("out", [cfg.G, cfg.NCLS], F32, kind="ExternalOutput").ap()

    with tile.TileContext(nc) as tc, \
            tc.tile_pool(name="plD", bufs=1, space="DRAM") as plD, \
            tc.tile_pool(name="plC", bufs=1) as plC:
        def DT(name, shape, addr_space="Local"):
            return plD.tile(shape, F32, tag=name, name=name,
                            addr_space=addr_space)

        def CT(name, shape, dtype=F32, src_ap=None):
            t = plC.tile(shape, dtype, tag=name, name=name)
            if src_ap is not None:
                nc.sync.dma_start(out=t[:], in_=src_ap)
            return t

        # shared DRAM scratch: per layer, 4 block buffers (block b = all
        # cores' quarter-b rows, rank-major within the block)
        h_blk = [[DT(f"hx{i}b{b}", [BSZ[b], HID], addr_space="Shared")
                  for b in range(NBLK)] for i in range(2)]
        agin = DT("agin", [SHARD, HID])
        ar_in = [DT(f"arin{i}", [1, 2 * HID]) for i in range(3)]
        ar_out = [DT(f"arout{i}", [1, 2 * HID], addr_space="Shared")
                  for i in range(3)]
        xag_in = DT("xagin", [P, HID])
        xag_out = DT("xagout", [NC_ * P, HID], addr_space="Shared")
        xt_dram = DT("xt_dram", [P, NT * HID])

        # ---------------- persistent SBUF ----------------
        idx_sb = CT("idx_sb", [P, (C * P) // 16], mybir.dt.int16, idx_d)
        dstloc_sb = CT("dstloc_sb", [P, C], F32, dstloc_d)
        gidloc_sb = CT("gidloc_sb", [P, NT], F32, gidloc_d)
        growid_sb = CT("growid_sb", [P, NC_], F32, growid_d)
        iota_sb = CT("iota_sb", [P, P], F32, iota_d)
        twoI_sb = CT("twoI_sb", [P, P], F32, twoI_d)
        ident_sb = CT("ident_sb", [P, P], F32, ident_d)
        ones_sb = CT("ones_sb", [P, P], F32, ones_d)

        W_sb = []  # per layer: list of [128, HID] k-half tiles
        for l in range(3):
            din = DIMS[l]
            halves = []
            for a in range((din + P - 1) // P):
                ka = min(P, din - a * P)
                wt = CT(f"Wsb{l}_{a}", [P, HID])
                nc.sync.dma_start(out=wt[:ka, :], in_=W_d[l][a * P:a * P + ka, :])
                halves.append(wt)
            W_sb.append(halves)
        bnp_sb = CT("bnp_sb", [1, 6 * HID], F32, bnp_d)

        fc1_sb = [CT(f"fc1sb{a}", [P, cfg.MLP0]) for a in range(HID // P)]
        for a in range(HID // P):
            nc.sync.dma_start(out=fc1_sb[a][:], in_=fc1_d[a * P:(a + 1) * P, :])
        bn1g_sb = CT("bn1g_sb", [P, cfg.MLP0 // P], F32, bn1g_d)
        bn1b_sb = CT("bn1b_sb", [P, cfg.MLP0 // P], F32, bn1b_d)
        mlpW_sb = [CT(f"mlpWsb{a}", [P, cfg.MLP1]) for a in range(cfg.MLP0 // P)]
        for a in range(cfg.MLP0 // P):
            nc.sync.dma_start(out=mlpW_sb[a][:], in_=mlpW_d[a * P:(a + 1) * P, :])
        mbng_sb = CT("mbng_sb", [P, cfg.MLP1 // P], F32, mbng_d)
        mbnb_sb = CT("mbnb_sb", [P, cfg.MLP1 // P], F32, mbnb_d)
        fc2_sb = [CT(f"fc2sb{a}", [P, cfg.NCLS]) for a in range(cfg.MLP1 // P)]
        for a in range(cfg.MLP1 // P):
            nc.sync.dma_start(out=fc2_sb[a][:], in_=fc2_d[a * P:(a + 1) * P, :])
        fc2b_sb = CT("fc2b_sb", [1, cfg.NCLS], F32, fc2b_d)

        cmax = max(int(c_tb.sum(axis=1).max()), 1)

        with (
            tc.tile_pool(name="plG", bufs=6) as plG,
            tc.tile_pool(name="plgs", bufs=3) as plgs,
            tc.tile_pool(name="plM", bufs=3) as plM,
            tc.tile_pool(name="plX", bufs=3) as plX,
            tc.tile_pool(name="plh", bufs=3) as plh,
            tc.tile_pool(name="plxt", bufs=3) as plxt,
            tc.tile_pool(name="plst", bufs=1) as plst,
            tc.tile_pool(name="plweff", bufs=2) as plweff,
            tc.tile_pool(name="pp", bufs=1, space="PSUM") as pp,
        ):
            for l in range(3):
                din = DIMS[l]
                dpad = cfg.DPAD if l == 0 else HID  # gather row width
                nh = (din + P - 1) // P  # k-halves
                hselfsrc = hself0 if l == 0 else agin
                lastcols = [min(P, din - a * P) for a in range(nh)]

                # per-layer persistent psum accumulators
                S_ps = [pp.tile([P, HID], F32, tag="Sps", bufs=3,
                                name=f"Sps{l}_{a}") for a in range(nh)]
                sumX_ps = pp.tile([P, HID], F32, tag="Sps", bufs=3,
                                  name=f"sumXps{l}") if nh < 3 else None
                if sumX_ps is None:
                    raise AssertionError("din>256 unsupported")

                g_tiles = {}
                # stream chunk s -> (group key, slot)
                chunk_grp = {}
                for bb in range(NBLK):
                    for gi in range((int(segcnt[bb]) + K - 1) // K):
                        s0g = int(seg0[bb]) + gi * K
                        kk = min(K, int(seg0[bb]) + int(segcnt[bb]) - s0g)
                        for sl in range(kk):
                            chunk_grp[s0g + sl] = (bb, gi, s0g, kk, sl)

                def get_group(s, dpad=dpad, l=l, g_tiles=g_tiles,
                              chunk_grp=chunk_grp):
                    bb, gi, s0g, kk, sl = chunk_grp[s]
                    gkey = (bb, gi)
                    if gkey not in g_tiles:
                        gt_ = plG.tile([P, K * HID], F32, tag="G", name="Gt")
                        if not gathers:
                            g_tiles[gkey] = gt_
                            return g_tiles[gkey], sl
                        if l == 0:
                            src_ap = h0p[BLKOFF[bb]:BLKOFF[bb] + BSZ[bb], :]
                        else:
                            src_ap = h_blk[l - 1][bb][:]
                        nc.gpsimd.dma_gather(
                            out_ap=gt_[:, :kk * dpad]
                                .rearrange("p (k d) -> p k d", d=dpad),
                            in_ap=src_ap,
                            idxs_ap=idx_sb[:, 8 * s0g:8 * (s0g + kk)],
                            num_idxs=P * kk,
                            num_idxs_reg=P * kk,
                            elem_size=dpad,
                            single_packet=False,
                        )
                        g_tiles[gkey] = gt_
                    return g_tiles[gkey], sl

                # ---------- pass 1 ----------
                for t in range(NT):
                    ct = int(c_tb[t].sum())
                    nodes_t = min(P, SHARD - t * P)
                    agg = pp.tile([P, HID], F32, tag="agg", bufs=3, name="agg")
                    gs = plgs.tile([P, HID], F32, tag="gs", name="gs")
                    if nodes_t < P:
                        nc.vector.memset(gs[:, :din], 0.0)
                    nc.sync.dma_start(
                        out=gs[:nodes_t, :din],
                        in_=hselfsrc[t * P:t * P + nodes_t, :din])
                    if ct > 0:
                        M = plM.tile([P, cmax * P], F32, tag="M", name="M")
                        jj0 = 0
                        for bb in range(NBLK):
                            ctb = int(c_tb[t][bb])
                            if ctb == 0:
                                continue
                            cb0 = int(colb0[t][bb])
                            nc.vector.tensor_tensor(
                                out=M[:, jj0 * P:(jj0 + ctb) * P]
                                    .rearrange("p (c f) -> p c f", f=P),
                                in0=iota_sb[:].rearrange("p (o f) -> p o f", o=1)
                                    .to_broadcast([P, ctb, P]),
                                in1=dstloc_sb[:, cb0:cb0 + ctb]
                                    .rearrange("p (c o) -> p c o", o=1)
                                    .to_broadcast([P, ctb, P]),
                                op=OP.is_equal,
                            )
                            jj0 += ctb
                        jj = 0
                        for bb in range(NBLK):
                            for j in range(int(c_tb[t][bb])):
                                s = int(colb0[t][bb]) + j
                                gt_, sl = get_group(s)
                                nc.tensor.matmul(
                                    out=agg[:, :din],
                                    lhsT=M[:, jj * P:(jj + 1) * P],
                                    rhs=gt_[:, sl * dpad:sl * dpad + din],
                                    start=(jj == 0), stop=False,
                                )
                                jj += 1
                    nc.tensor.matmul(
                        out=agg[:, :din], lhsT=twoI_sb[:], rhs=gs[:, :din],
                        start=(ct == 0), stop=True,
                    )
                    X = plX.tile([P, HID], F32, tag="X", name="X")
                    nc.vector.tensor_copy(out=X[:, :din], in_=agg[:, :din])
                    # stats
                    nc.tensor.matmul(out=sumX_ps[0:1, :din], lhsT=ones_sb[:, 0:1],
                                     rhs=X[:, :din], start=(t == 0),
                                     stop=(t == NT - 1))
                    for a in range(nh):
                        ka = lastcols[a]
                        nc.tensor.matmul(
                            out=S_ps[a][:ka, :din], lhsT=X[:, a * P:a * P + ka],
                            rhs=X[:, :din], start=(t == 0), stop=(t == NT - 1))
                    # transpose X -> XT, spill to DRAM
                    xt_ps = pp.tile([P, HID], F32, tag="xt", bufs=2, name="xtps")
                    for a in range(nh):
                        ka = lastcols[a]
                        nc.tensor.transpose(
                            out=xt_ps[:ka, a * P:a * P + P],
                            in_=X[:, a * P:a * P + ka],
                            identity=ident_sb[:],
                        )
                    xt_sb = plxt.tile([P, HID], F32, tag="xtsb", name="xtsb")
                    for a in range(nh):
                        ka = lastcols[a]
                        nc.scalar.copy(out=xt_sb[:ka, a * P:a * P + P],
                                       in_=xt_ps[:ka, a * P:a * P + P])
                    kmax = min(P, din)
                    nc.scalar.dma_start(
                        out=xt_dram[:kmax, t * HID:t * HID + nh * P],
                        in_=xt_sb[:kmax, :nh * P])

                # ---------- per-layer stats & BN folding ----------
                S_sb = []
                for a in range(nh):
                    ka = lastcols[a]
                    st = plst.tile([P, HID], F32, tag=f"Ssb{a}", name=f"Ssb{a}")
                    nc.scalar.copy(out=st[:ka, :din], in_=S_ps[a][:ka, :din])
                    S_sb.append(st)
                sumX_sb = plst.tile([1, HID], F32, tag="sumXsb", name="sumXsb")
                nc.scalar.copy(out=sumX_sb[:, :din], in_=sumX_ps[0:1, :din])
                # sumY2 = colsum(W * (S @ W))   (S symmetric)
                y2s_ps = pp.tile([P, HID], F32, tag="agg", bufs=3, name="y2sps")
                for b in range(nh):
                    kb = lastcols[b]
                    SW_ps = pp.tile([P, HID], F32, tag="xt", bufs=2, name="SWps")
                    for a in range(nh):
                        ka = lastcols[a]
                        nc.tensor.matmul(
                            out=SW_ps[:kb, :],
                            lhsT=S_sb[a][:ka, b * P:b * P + kb],
                            rhs=W_sb[l][a][:ka, :],
                            start=(a == 0), stop=(a == nh - 1))
                    wsw = plst.tile([P, HID], F32, tag="wsw", name="wsw")
                    nc.vector.tensor_mul(out=wsw[:kb, :], in0=W_sb[l][b][:kb, :],
                                         in1=SW_ps[:kb, :])
                    nc.tensor.matmul(out=y2s_ps[0:1, :], lhsT=ones_sb[:kb, 0:1],
                                     rhs=wsw[:kb, :], start=(b == 0),
                                     stop=(b == nh - 1))
                # sumY = sumX @ W : transpose sumX, then matmul
                sxc = plst.tile([P, 2], F32, tag="sxc", name="sxc")
                for a in range(nh):
                    ka = lastcols[a]
                    tp = pp.tile([P, HID], F32, tag="xt", bufs=2, name="tp1")
                    nc.tensor.transpose(out=tp[:ka, 0:1],
                                        in_=sumX_sb[0:1, a * P:a * P + ka],
                                        identity=ident_sb[0:1, 0:1])
                    nc.scalar.copy(out=sxc[:ka, a:a + 1], in_=tp[:ka, 0:1])
                sy_ps = pp.tile([P, HID], F32, tag="xt", bufs=2, name="syps")
                for a in range(nh):
                    ka = lastcols[a]
                    nc.tensor.matmul(out=sy_ps[0:1, :], lhsT=sxc[:ka, a:a + 1],
                                     rhs=W_sb[l][a][:ka, :], start=(a == 0),
                                     stop=(a == nh - 1))
                arp = plst.tile([1, 2 * HID], F32, tag="arp", name="arp")
                nc.scalar.copy(out=arp[0:1, :HID], in_=sy_ps[0:1, :])
                nc.scalar.copy(out=arp[0:1, HID:], in_=y2s_ps[0:1, :])
                nc.sync.dma_start(out=ar_in[l][:], in_=arp[:])
                if coll:
                    nc.gpsimd.collective_compute(
                        "AllReduce", OP.add,
                        replica_groups=[list(range(NC_))],
                        ins=[ar_in[l][:]], outs=[ar_out[l][:]],
                    )
                else:
                    nc.sync.dma_start(out=ar_out[l][:], in_=ar_in[l][:])
                arr = plst.tile([1, 2 * HID], F32, tag="arr", name="arr")
                nc.sync.dma_start(out=arr[:], in_=ar_out[l][:])
                mean = plst.tile([1, HID], F32, tag="mean", name="mean")
                nc.scalar.mul(out=mean[:], in_=arr[0:1, :HID], mul=1.0 / N)
                ex2 = plst.tile([1, HID], F32, tag="stmp0", name="ex2")
                nc.scalar.mul(out=ex2[:], in_=arr[0:1, HID:], mul=1.0 / N)
                msq = plst.tile([1, HID], F32, tag="stmp1", name="msq")
                nc.scalar.activation(out=msq[:], in_=mean[:], func=AF.Square)
                var = plst.tile([1, HID], F32, tag="stmp2", name="var")
                nc.vector.tensor_sub(out=var[:], in0=ex2[:], in1=msq[:])
                vep = plst.tile([1, HID], F32, tag="stmp0", name="vep")
                nc.vector.tensor_scalar_add(out=vep[:], in0=var[:], scalar1=EPS_BN)
                rv = plst.tile([1, HID], F32, tag="stmp1", name="rv")
                nc.vector.reciprocal(out=rv[:], in_=vep[:])
                rs = plst.tile([1, HID], F32, tag="stmp2", name="rs")
                nc.scalar.activation(out=rs[:], in_=rv[:], func=AF.Sqrt)
                A_sb = plst.tile([1, HID], F32, tag="A_sb", name="A_sb")
                nc.vector.tensor_mul(out=A_sb[:], in0=rs[:], in1=bnp_sb[0:1, l * HID:(l + 1) * HID])
                mA = plst.tile([1, HID], F32, tag="stmp0", name="mA")
                nc.vector.tensor_mul(out=mA[:], in0=mean[:], in1=A_sb[:])
                B_sb = plst.tile([1, HID], F32, tag="B_sb", name="B_sb")
                nc.vector.tensor_sub(out=B_sb[:], in0=bnp_sb[0:1, (3 + l) * HID:(4 + l) * HID],
                                     in1=mA[:])
                arep_ps = pp.tile([P, HID], F32, tag="xt", bufs=2, name="arepps")
                nc.tensor.matmul(out=arep_ps[:], lhsT=ones_sb[0:1, :],
                                 rhs=A_sb[:], start=True, stop=True)
                Weff = []
                for a in range(nh):
                    ka = lastcols[a]
                    we = plweff.tile([P, HID], F32, tag="weff", name="weff")
                    nc.vector.tensor_mul(out=we[:ka, :], in0=W_sb[l][a][:ka, :],
                                         in1=arep_ps[:ka, :])
                    Weff.append(we)

                # ---------- pass 2 ----------
                if l == 2:
                    xread_ps = pp.tile([P, HID], F32, tag="Sps", bufs=3,
                                       name="xreadps")
                for t in range(NT):
                    nodes_t = min(P, SHARD - t * P)
                    kmax = min(P, din)
                    xt_ld = plxt.tile([P, HID], F32, tag="xtld", name="xtld")
                    nc.scalar.dma_start(out=xt_ld[:kmax, :nh * P],
                                        in_=xt_dram[:kmax, t * HID:t * HID + nh * P])
                    y2 = pp.tile([P, max(GQ, HID)], F32, tag="xt", bufs=2,
                                 name="y2")
                    for a in range(nh):
                        ka = lastcols[a]
                        nc.tensor.matmul(
                            out=y2[:, :HID],
                            lhsT=xt_ld[:ka, a * P:a * P + P],
                            rhs=Weff[a][:ka, :], start=(a == 0), stop=False)
                    nc.tensor.matmul(out=y2[:, :HID], lhsT=ones_sb[0:1, :],
                                     rhs=B_sb[:], start=False, stop=True)
                    hnew = plh.tile([P, HID], F32, tag="hnew", name="hnew")
                    nc.scalar.activation(out=hnew[:], in_=y2[:, :HID], func=AF.Relu)
                    if l < 2:
                        nc.sync.dma_start(
                            out=agin[t * P:t * P + nodes_t, :],
                            in_=hnew[:nodes_t, :])
                        qends = np.cumsum(TQ) - 1
                        if t in qends:
                            qq = int(np.where(qends == t)[0][0])
                            if coll:
                                nc.gpsimd.collective_compute(
                                    "AllGather", OP.bypass,
                                    replica_groups=[list(range(NC_))],
                                    ins=[agin[QOFF[qq]:QOFF[qq] + QB[qq], :]],
                                    outs=[h_blk[l][qq][:]],
                                )
                            else:
                                nc.sync.dma_start(
                                    out=h_blk[l][qq][0:QB[qq], :],
                                    in_=agin[QOFF[qq]:QOFF[qq] + QB[qq], :])
                    else:
                        R = plM.tile([P, P], F32, tag="R", name="R")
                        nc.vector.tensor_tensor(
                            out=R[:], in0=iota_sb[:],
                            in1=gidloc_sb[:, t:t + 1].to_broadcast([P, P]),
                            op=OP.is_equal)
                        nc.tensor.matmul(out=xread_ps[:], lhsT=R[:], rhs=hnew[:],
                                         start=(t == 0), stop=(t == NT - 1))

            # ---------------- readout exchange ----------------
            xp_sb = plst.tile([P, HID], F32, tag="xp_sb", name="xp_sb")
            nc.scalar.copy(out=xp_sb[:], in_=xread_ps[:])
            nc.sync.dma_start(out=xag_in[:], in_=xp_sb[:])
            if coll:
                nc.gpsimd.collective_compute(
                    "AllGather", OP.bypass,
                    replica_groups=[list(range(NC_))],
                    ins=[xag_in[:]], outs=[xag_out[:]],
                )
            else:
                nc.sync.dma_start(out=xag_out[0:P, :], in_=xag_in[:])
            xg_sb = plst.tile([P, NC_ * HID], F32, tag="xg_sb", name="xg_sb")
            nc.sync.dma_start(
                out=xg_sb[:].rearrange("p (c f) -> p c f", f=HID),
                in_=xag_out[:].rearrange("(c p) f -> p c f", p=P))
            xfull_sb = []
            for gt in range(GT):
                xf_ps = pp.tile([P, HID], F32, tag="agg", bufs=3, name="xfps")
                for c in range(NC_):
                    gsh = plM.tile([P, 1], F32, tag="gsh", name="gsh")
                    nc.vector.tensor_scalar(
                        out=gsh[:], in0=growid_sb[:, c:c + 1],
                        scalar1=float(P * gt), scalar2=None, op0=OP.subtract)
                    R2 = plM.tile([P, P], F32, tag="R", name="R2")
                    nc.vector.tensor_tensor(
                        out=R2[:], in0=iota_sb[:],
                        in1=gsh[:].to_broadcast([P, P]), op=OP.is_equal)
                    nc.tensor.matmul(out=xf_ps[:],
                                     lhsT=R2[:],
                                     rhs=xg_sb[:, c * HID:(c + 1) * HID],
                                     start=(c == 0), stop=(c == NC_ - 1))
                xf = plst.tile([P, HID], F32, tag=f"xfull{gt}", name=f"xfull{gt}")
                nc.scalar.copy(out=xf[:], in_=xf_ps[:])
                xfull_sb.append(xf)
            # transpose -> xT [HID(2 tiles), GQ]
            xT_sb = [plst.tile([P, GQ], F32, tag=f"xT{h}", name=f"xT{h}")
                     for h in range(HID // P)]
            for h in range(HID // P):
                for gt in range(GT):
                    tp2 = pp.tile([P, HID], F32, tag="xt", bufs=2, name="tp2")
                    nc.tensor.transpose(out=tp2[:, :P],
                                        in_=xfull_sb[gt][:, h * P:(h + 1) * P],
                                        identity=ident_sb[:])
                    nc.scalar.copy(out=xT_sb[h][:, gt * P:(gt + 1) * P],
                                   in_=tp2[:, :P])

            def bn_relu_T(y_ps, g_ap, b_ap, nb, tagsfx):
                """BN(train)+relu on feature-major psum tile [128, nb]."""
                s1 = plst.tile([P, 1], F32, tag="s1", name="s1")
                nc.vector.tensor_reduce(out=s1[:], in_=y_ps[:, :nb],
                                        axis=AX.X, op=OP.add)
                sq = plst.tile([P, GQ], F32, tag="sq", name="sq")
                s2 = plst.tile([P, 1], F32, tag="s2", name="s2")
                nc.scalar.activation(out=sq[:, :nb], in_=y_ps[:, :nb],
                                     func=AF.Square, accum_out=s2[:])
                mn = plst.tile([P, 1], F32, tag="mn", name="mn")
                nc.scalar.mul(out=mn[:], in_=s1[:], mul=1.0 / nb)
                e2 = plst.tile([P, 1], F32, tag="e2", name="e2")
                nc.scalar.mul(out=e2[:], in_=s2[:], mul=1.0 / nb)
                ms = plst.tile([P, 1], F32, tag="ms", name="ms")
                nc.scalar.activation(out=ms[:], in_=mn[:], func=AF.Square)
                vr = plst.tile([P, 1], F32, tag="vr", name="vr")
                nc.vector.tensor_sub(out=vr[:], in0=e2[:], in1=ms[:])
                ve = plst.tile([P, 1], F32, tag="ve", name="ve")
                nc.vector.tensor_scalar_add(out=ve[:], in0=vr[:], scalar1=EPS_BN)
                rv2 = plst.tile([P, 1], F32, tag="rv2", name="rv2")
                nc.vector.reciprocal(out=rv2[:], in_=ve[:])
                rs2 = plst.tile([P, 1], F32, tag="rs2", name="rs2")
                nc.scalar.activation(out=rs2[:], in_=rv2[:], func=AF.Sqrt)
                Am = plst.tile([P, 1], F32, tag="Am", name="Am")
                nc.vector.tensor_mul(out=Am[:], in0=rs2[:], in1=g_ap)
                mAm = plst.tile([P, 1], F32, tag="mAm", name="mAm")
                nc.vector.tensor_mul(out=mAm[:], in0=mn[:], in1=Am[:])
                Bm = plst.tile([P, 1], F32, tag="Bm", name="Bm")
                nc.vector.tensor_sub(out=Bm[:], in0=b_ap, in1=mAm[:])
                yo = plst.tile([P, GQ], F32, tag="yo", bufs=6, name=f"yo{tagsfx}")
                if nb < GQ:
                    nc.vector.memset(yo[:], 0.0)
                nc.scalar.activation(out=yo[:, :nb], in_=y_ps[:, :nb], func=AF.Relu,
                                     scale=Am[:], bias=Bm[:])
                return yo

            # FC1 -> BN -> relu (feature-major), then hidden MLP, then FC2
            y1t = []
            for m in range(cfg.MLP0 // P):
                y1_ps = pp.tile([P, max(GQ, HID)], F32, tag="xt", bufs=2,
                                name="y1ps")
                for k in range(HID // P):
                    nc.tensor.matmul(out=y1_ps[:, :GQ],
                                     lhsT=fc1_sb[k][:, m * P:(m + 1) * P],
                                     rhs=xT_sb[k][:], start=(k == 0),
                                     stop=(k == HID // P - 1))
                y1t.append(bn_relu_T(y1_ps, bn1g_sb[:, m:m + 1],
                                     bn1b_sb[:, m:m + 1], cfg.G, f"a{m}"))
            y2t = []
            for m in range(cfg.MLP1 // P):
                y2_ps = pp.tile([P, max(GQ, HID)], F32, tag="xt", bufs=2,
                                name="y2ps")
                for k in range(cfg.MLP0 // P):
                    nc.tensor.matmul(out=y2_ps[:, :GQ],
                                     lhsT=mlpW_sb[k][:, m * P:(m + 1) * P],
                                     rhs=y1t[k][:], start=(k == 0),
                                     stop=(k == cfg.MLP0 // P - 1))
                y2t.append(bn_relu_T(y2_ps, mbng_sb[:, m:m + 1],
                                     mbnb_sb[:, m:m + 1], cfg.G, f"b{m}"))
            for gt in range(GT):
                ng = min(P, cfg.G - gt * P)
                lg_ps = pp.tile([P, HID], F32, tag="agg", bufs=3, name="lgps")
                for k in range(cfg.MLP1 // P):
                    nc.tensor.matmul(out=lg_ps[:, :cfg.NCLS],
                                     lhsT=y2t[k][:, gt * P:gt * P + P],
                                     rhs=fc2_sb[k][:], start=(k == 0), stop=False)
                nc.tensor.matmul(out=lg_ps[:, :cfg.NCLS], lhsT=ones_sb[0:1, :],
                                 rhs=fc2b_sb[:], start=False, stop=True)
                ot = plh.tile([P, cfg.NCLS], F32, tag="ot", name="ot")
                nc.scalar.activation(out=ot[:ng, :], in_=lg_ps[:ng, :cfg.NCLS],
                                     func=AF.Sigmoid)
                nc.sync.dma_start(out=out_d[gt * P:gt * P + ng, :], in_=ot[:ng, :])


def _pack_inputs(inputs, cfg, meta):
    """Build per-core in_maps."""
    NC_ = cfg.NC
    h = np.ascontiguousarray(np.asarray(inputs["h"], np.float32))
    h0p = np.zeros((cfg.N, cfg.DPAD), np.float32)
    perm = np.empty(cfg.N, np.int64)
    pos = 0
    for b in range(cfg.NBLK):
        for c in range(NC_):
            s0 = c * cfg.SHARD + cfg.QOFF[b]
            perm[pos:pos + cfg.QB[b]] = np.arange(s0, s0 + cfg.QB[b])
            pos += cfg.QB[b]
    h0p[:, :cfg.IN_FEATS] = h[perm]
    iota = np.ascontiguousarray(
        np.tile(np.arange(P, dtype=np.float32)[None, :], (P, 1)))
    twoI = np.ascontiguousarray(2.0 * np.eye(P, dtype=np.float32))
    ident = np.ascontiguousarray(np.eye(P, dtype=np.float32))
    ones = np.ones((P, P), np.float32)
    Ws = [np.ascontiguousarray(np.asarray(w, np.float32)) for w in inputs["gcn_Ws"]]
    bnp = np.ascontiguousarray(np.concatenate(
        [np.asarray(x, np.float32) for x in inputs["bn_gs"]] +
        [np.asarray(x, np.float32) for x in inputs["bn_bs"]])[None, :])
    fc1W = np.ascontiguousarray(np.asarray(inputs["fc1_W"], np.float32))
    bn1g = np.ascontiguousarray(
        np.asarray(inputs["bn1_g"], np.float32).reshape(-1, P).T)
    bn1b = np.ascontiguousarray(
        np.asarray(inputs["bn1_b"], np.float32).reshape(-1, P).T)
    mlpW = np.ascontiguousarray(np.asarray(inputs["mlp_Ws"][0], np.float32))
    mbng = np.ascontiguousarray(
        np.asarray(inputs["mlp_bn_gs"][0], np.float32).reshape(-1, P).T)
    mbnb = np.ascontiguousarray(
        np.asarray(inputs["mlp_bn_bs"][0], np.float32).reshape(-1, P).T)
    fc2W = np.ascontiguousarray(
        np.asarray(inputs["fc2_W"], np.float32)[:, -cfg.NCLS:])
    fc2b = np.ascontiguousarray(
        np.asarray(inputs["fc2_b"], np.float32)[None, -cfg.NCLS:])
    in_maps = []
    for c in range(NC_):
        in_maps.append(dict(
            h0p=h0p,
            hself0=np.ascontiguousarray(h[c * cfg.SHARD:(c + 1) * cfg.SHARD]),
            idx16=np.ascontiguousarray(meta["idx16"][c]),
            dstloc=np.ascontiguousarray(meta["dstloc"][c]),
            gidloc=np.ascontiguousarray(meta["gidloc"][c]),
            growid=np.ascontiguousarray(meta["growid"]),
            iota=iota, twoI=twoI, ident=ident, ones=ones,
            W0=Ws[0], W1=Ws[1], W2=Ws[2], bnp=bnp,
            fc1W=fc1W, bn1gT=bn1g, bn1bT=bn1b,
            mlpW=mlpW, mbngT=mbng, mbnbT=mbnb,
            fc2Wr=fc2W, fc2br=fc2b,
        ))
    return in_maps


def make_nc(cfg, meta, coll=True, gathers=True, num_devices=None):
    nc = bacc.Bacc("TRN2", target_bir_lowering=False, debug=False,
                   enable_asserts=False,
                   num_devices=num_devices or (cfg.NC if coll else 1))
    _build(nc, cfg, meta, coll=coll, gathers=gathers)
    nc.compile()
    return nc


def build_and_run(inputs, cfg, **run_kwargs):
    meta = _plan(inputs["src"], inputs["dst"], inputs["graph_id"], cfg)
    nc = make_nc(cfg, meta)
    in_maps = _pack_inputs(inputs, cfg, meta)
    res = run_bass_kernel_spmd(nc, in_maps, core_ids=list(range(cfg.NC)),
                               **run_kwargs)
    return res


def kernel(**inputs):
    cfg = Cfg()
    res = build_and_run(inputs, cfg)
    return np.asarray(res.results[0]["out"], np.float32)


# revision 22
# speedup vs baseline: 1.0397x; 1.0397x over previous
"""GIN message-passing classifier on 8 Trainium2 NeuronCores.

Strategy (graph/node partition, data parallel):
  - Nodes are split into 8 equal contiguous shards (12500 nodes/core); each
    core owns the edges whose *destination* lands in its shard.
  - Host pre-sorts edges by dst, groups them per 128-node tile, pads each
    tile's edge list to a multiple of 128 ("chunks").  Chunk counts are taken
    as the max over cores so all 8 cores run one identical program (SPMD).
  - On device, chunks of 128 edges are gathered with batched indirect DMA
    (h[src] rows) and scatter-added into the tile's PSUM accumulator with a
    one-hot selector matmul built on the fly by the vector engine (is_equal
    against an iota row).  The GIN self term (2*h) is one extra matmul with a
    constant 2*I selector on contiguously-loaded own rows.
  - BatchNorm batch statistics come from per-core partials (sum(X) and the
    second moment X^T X pushed through W analytically), combined with a tiny
    [2,256] AllReduce; scale/shift are folded into W on device, so the
    per-node epilogue is matmul + fused relu.
  - h_new is exchanged between layers with an AllGather (rows = node shards).
  - Readout (segment-sum per graph) uses the same one-hot-selector matmul
    into per-core local graph slots, a small AllGather, and a host-planned
    slot->graph selector reduce; the graph-level MLP is replicated.
"""

import numpy as np

import concourse.bass as bass
import concourse.mybir as mybir
import concourse.tile as tile
from concourse import bacc
from concourse.bass_utils import run_bass_kernel_spmd

P = 128


def _T(tc, *args, **kw):
    t, _free = tc.tile(*args, **kw)
    return t


F32 = mybir.dt.float32
I32 = mybir.dt.int32
AF = mybir.ActivationFunctionType
OP = mybir.AluOpType
AX = mybir.AxisListType
EPS_BN = 1e-5


class Cfg:
    def __init__(self, N=100000, E=1000000, G=512, IN_FEATS=78, HID=256,
                 MLP0=512, MLP1=256, NCLS=204, NC=8, K_GATHER=8):
        self.N, self.E, self.G = N, E, G
        self.IN_FEATS, self.HID = IN_FEATS, HID
        self.MLP0, self.MLP1, self.NCLS = MLP0, MLP1, NCLS
        self.NC, self.K = NC, K_GATHER
        assert N % NC == 0
        self.SHARD = N // NC
        self.NT = (self.SHARD + P - 1) // P
        self.GT = (G + P - 1) // P
        self.NBLK = 4
        # block b = all cores' quarter-b of their shard; quarters are
        # tile-aligned so pipelined AllGathers unblock gather segments.
        tq = (self.NT + 3) // 4           # tiles per quarter (last smaller)
        self.TQ = [tq, tq, tq, self.NT - 3 * tq]
        qb = [min(t * P, self.SHARD) for t in np.cumsum([0] + self.TQ)]
        self.QOFF = qb[:4]                # row offset of quarter q in shard
        self.QB = [qb[i + 1] - qb[i] for i in range(4)]  # rows per quarter
        self.BSZ = [self.NC * q for q in self.QB]        # rows per block
        self.BLKOFF = [0]
        for b in range(3):
            self.BLKOFF.append(self.BLKOFF[-1] + self.BSZ[b])
        assert max(self.BSZ) < 32768, "dma_gather int16 index range"
        self.DPAD = 128  # layer-0 gather row padded to 128 f32 (512B)


def _plan(src, dst, graph_id, cfg):
    """Host-side edge bucketing. Returns per-core data + shared metadata.

    Edges are bucketed by (dst-tile, src-block); the chunk stream is laid out
    block-major (all tiles' block-0 chunks, then block-1, ...) so each
    dma_gather instruction covers one contiguous same-block run of chunks.
    """
    NC, SHARD, NT = cfg.NC, cfg.SHARD, cfg.NT
    NBLK = cfg.NBLK
    qoff = np.asarray(cfg.QOFF + [SHARD], np.int64)
    qb = np.asarray(cfg.QB, np.int64)
    src = np.asarray(src).astype(np.int64).ravel()
    dst = np.asarray(dst).astype(np.int64).ravel()
    gid = np.asarray(graph_id).astype(np.int64).ravel()

    core = dst // SHARD
    rem = dst % SHARD
    t = rem // P
    loc = rem % P
    score = src // SHARD
    soff = src % SHARD
    b = np.searchsorted(qoff, soff, side="right") - 1
    key = (core * NT + t) * NBLK + b
    order = np.argsort(key, kind="stable")
    key = key[order]
    srcl = (score * qb[b] + (soff - qoff[b]))[order]
    assert srcl.max() < 32768
    loc = loc[order]
    cnt = np.bincount(key, minlength=NC * NT * NBLK).reshape(NC, NT, NBLK)
    c_tb = ((cnt + P - 1) // P).max(axis=0).astype(np.int64)  # [NT, NBLK]
    segcnt = c_tb.sum(axis=0)                                  # [NBLK]
    seg0 = np.zeros(NBLK, np.int64)
    seg0[1:] = np.cumsum(segcnt)[:-1]
    # chunk-stream start of each (t, b) run
    colb0 = np.zeros((NT, NBLK), np.int64)
    for bb in range(NBLK):
        colb0[0, bb] = seg0[bb]
        colb0[1:, bb] = seg0[bb] + np.cumsum(c_tb[:-1, bb])
    C = max(int(c_tb.sum()), 1)

    starts = np.zeros(NC * NT * NBLK + 1, np.int64)
    starts[1:] = np.cumsum(cnt.ravel())
    rank = np.arange(len(key)) - starts[key]
    ch = rank // P
    p = rank % P
    tt = (key // NBLK) % NT
    bb_ = key % NBLK
    cc = key // (NT * NBLK)
    s = colb0[tt, bb_] + ch  # stream chunk index
    g = s * P + p            # global slot position

    idx16 = np.zeros((NC, C * P), np.int16)
    dstloc = np.full((NC, P, C), -1.0, np.float32)
    idx16[cc, g] = srcl.astype(np.int16)
    dstloc[cc, p, s] = loc.astype(np.float32)
    # wrapped layout: position g -> [g % 16, g // 16], replicated on 8 Q7 cores
    idx16w = np.zeros((NC, P, (C * P) // 16), np.int16)
    for c in range(NC):
        w = idx16[c].reshape((C * P) // 16, 16).T  # [16, 8C]
        idx16w[c] = np.tile(w, (8, 1))

    gidloc = np.full((NC, P, NT), -1.0, np.float32)
    growid = np.full((P, NC), -1.0, np.float32)
    for c in range(NC):
        gg = gid[c * SHARD:(c + 1) * SHARD]
        gb, gm = int(gg[0]), int(gg[-1])
        assert gm - gb < P, "graph span exceeds 128 per core"
        arr = np.full(NT * P, -1.0, np.float32)
        arr[:SHARD] = (gg - gb).astype(np.float32)
        gidloc[c] = arr.reshape(NT, P).T
        jj = np.arange(P)
        sel = (gb + jj) <= gm
        growid[sel, c] = (gb + jj[sel]).astype(np.float32)

    return dict(c_tb=c_tb, C=C, colb0=colb0, seg0=seg0, segcnt=segcnt,
                idx16=idx16w, dstloc=dstloc, gidloc=gidloc, growid=growid)


def _build(nc, cfg, meta, coll=True, gathers=True):
    """Trace the full Bass/Tile program (shared by all 8 cores)."""
    NT, C, K = cfg.NT, meta["C"], cfg.K
    c_tb, colb0 = meta["c_tb"], meta["colb0"]
    seg0, segcnt = meta["seg0"], meta["segcnt"]
    NBLK = cfg.NBLK
    BSZ, BLKOFF, QOFF, QB, TQ = cfg.BSZ, cfg.BLKOFF, cfg.QOFF, cfg.QB, cfg.TQ
    HID = cfg.HID
    DIMS = [cfg.IN_FEATS, HID, HID]
    N, SHARD, NC_ = cfg.N, cfg.SHARD, cfg.NC
    GT = cfg.GT
    GQ = GT * P

    # ---------------- DRAM I/O ----------------
    h0p = nc.dram_tensor("h0p", [N, cfg.DPAD], F32, kind="ExternalInput").ap()
    hself0 = nc.dram_tensor("hself0", [SHARD, cfg.IN_FEATS], F32,
                            kind="ExternalInput").ap()
    idx_d = nc.dram_tensor("idx16", [P, (C * P) // 16], mybir.dt.int16,
                           kind="ExternalInput").ap()
    dstloc_d = nc.dram_tensor("dstloc", [P, C], F32, kind="ExternalInput").ap()
    gidloc_d = nc.dram_tensor("gidloc", [P, NT], F32, kind="ExternalInput").ap()
    growid_d = nc.dram_tensor("growid", [P, NC_], F32, kind="ExternalInput").ap()
    iota_d = nc.dram_tensor("iota", [P, P], F32, kind="ExternalInput").ap()
    twoI_d = nc.dram_tensor("twoI", [P, P], F32, kind="ExternalInput").ap()
    ident_d = nc.dram_tensor("ident", [P, P], F32, kind="ExternalInput").ap()
    ones_d = nc.dram_tensor("ones", [P, P], F32, kind="ExternalInput").ap()
    W_d = [nc.dram_tensor(f"W{i}", [DIMS[i], HID], F32, kind="ExternalInput").ap()
           for i in range(3)]
    bnp_d = nc.dram_tensor("bnp", [1, 6 * HID], F32,
                           kind="ExternalInput").ap()
    fc1_d = nc.dram_tensor("fc1W", [HID, cfg.MLP0], F32, kind="ExternalInput").ap()
    bn1g_d = nc.dram_tensor("bn1gT", [P, cfg.MLP0 // P], F32,
                            kind="ExternalInput").ap()
    bn1b_d = nc.dram_tensor("bn1bT", [P, cfg.MLP0 // P], F32,
                            kind="ExternalInput").ap()
    mlpW_d = nc.dram_tensor("mlpW", [cfg.MLP0, cfg.MLP1], F32,
                            kind="ExternalInput").ap()
    mbng_d = nc.dram_tensor("mbngT", [P, cfg.MLP1 // P], F32,
                            kind="ExternalInput").ap()
    mbnb_d = nc.dram_tensor("mbnbT", [P, cfg.MLP1 // P], F32,
                            kind="ExternalInput").ap()
    fc2_d = nc.dram_tensor("fc2Wr", [cfg.MLP1, cfg.NCLS], F32,
                           kind="ExternalInput").ap()
    fc2b_d = nc.dram_tensor("fc2br", [1, cfg.NCLS], F32, kind="ExternalInput").ap()
    out_d = nc.dram_tensor("out", [cfg.G, cfg.NCLS], F32, kind="ExternalOutput").ap()

    with tile.TileContext(nc) as tc, \
            tc.tile_pool(name="plD", bufs=1, space="DRAM") as plD, \
            tc.tile_pool(name="plC", bufs=1) as plC:
        def DT(name, shape, addr_space="Local"):
            return plD.tile(shape, F32, tag=name, name=name,
                            addr_space=addr_space)

        def CT(name, shape, dtype=F32, src_ap=None):
            t = plC.tile(shape, dtype, tag=name, name=name)
            if src_ap is not None:
                nc.sync.dma_start(out=t[:], in_=src_ap)
            return t

        # shared DRAM scratch: per layer, 4 block buffers (block b = all
        # cores' quarter-b rows, rank-major within the block)
        h_blk = [[DT(f"hx{i}b{b}", [BSZ[b], HID], addr_space="Shared")
                  for b in range(NBLK)] for i in range(2)]
        agin = DT("agin", [SHARD, HID])
        ar_in = [DT(f"arin{i}", [1, 2 * HID]) for i in range(3)]
        ar_out = [DT(f"arout{i}", [1, 2 * HID], addr_space="Shared")
                  for i in range(3)]
        xag_in = DT("xagin", [P, HID])
        xag_out = DT("xagout", [NC_ * P, HID], addr_space="Shared")
        xt_dram = DT("xt_dram", [P, NT * HID])

        # ---------------- persistent SBUF ----------------
        idx_sb = CT("idx_sb", [P, (C * P) // 16], mybir.dt.int16, idx_d)
        dstloc_sb = CT("dstloc_sb", [P, C], F32, dstloc_d)
        gidloc_sb = CT("gidloc_sb", [P, NT], F32, gidloc_d)
        growid_sb = CT("growid_sb", [P, NC_], F32, growid_d)
        iota_sb = CT("iota_sb", [P, P], F32, iota_d)
        twoI_sb = CT("twoI_sb", [P, P], F32, twoI_d)
        ident_sb = CT("ident_sb", [P, P], F32, ident_d)
        ones_sb = CT("ones_sb", [P, P], F32, ones_d)

        W_sb = []  # per layer: list of [128, HID] k-half tiles
        for l in range(3):
            din = DIMS[l]
            halves = []
            for a in range((din + P - 1) // P):
                ka = min(P, din - a * P)
                wt = CT(f"Wsb{l}_{a}", [P, HID])
                nc.sync.dma_start(out=wt[:ka, :], in_=W_d[l][a * P:a * P + ka, :])
                halves.append(wt)
            W_sb.append(halves)
        bnp_sb = CT("bnp_sb", [1, 6 * HID], F32, bnp_d)

        fc1_sb = [CT(f"fc1sb{a}", [P, cfg.MLP0]) for a in range(HID // P)]
        for a in range(HID // P):
            nc.sync.dma_start(out=fc1_sb[a][:], in_=fc1_d[a * P:(a + 1) * P, :])
        bn1g_sb = CT("bn1g_sb", [P, cfg.MLP0 // P], F32, bn1g_d)
        bn1b_sb = CT("bn1b_sb", [P, cfg.MLP0 // P], F32, bn1b_d)
        mlpW_sb = [CT(f"mlpWsb{a}", [P, cfg.MLP1]) for a in range(cfg.MLP0 // P)]
        for a in range(cfg.MLP0 // P):
            nc.sync.dma_start(out=mlpW_sb[a][:], in_=mlpW_d[a * P:(a + 1) * P, :])
        mbng_sb = CT("mbng_sb", [P, cfg.MLP1 // P], F32, mbng_d)
        mbnb_sb = CT("mbnb_sb", [P, cfg.MLP1 // P], F32, mbnb_d)
        fc2_sb = [CT(f"fc2sb{a}", [P, cfg.NCLS]) for a in range(cfg.MLP1 // P)]
        for a in range(cfg.MLP1 // P):
            nc.sync.dma_start(out=fc2_sb[a][:], in_=fc2_d[a * P:(a + 1) * P, :])
        fc2b_sb = CT("fc2b_sb", [1, cfg.NCLS], F32, fc2b_d)

        cmax = max(int(c_tb.sum(axis=1).max()), 1)

        with (
            tc.tile_pool(name="plG", bufs=7) as plG,
            tc.tile_pool(name="plgs", bufs=3) as plgs,
            tc.tile_pool(name="plM", bufs=2) as plM,
            tc.tile_pool(name="plX", bufs=3) as plX,
            tc.tile_pool(name="plh", bufs=3) as plh,
            tc.tile_pool(name="plxt", bufs=3) as plxt,
            tc.tile_pool(name="plst", bufs=1) as plst,
            tc.tile_pool(name="plweff", bufs=2) as plweff,
            tc.tile_pool(name="pp", bufs=1, space="PSUM") as pp,
        ):
            for l in range(3):
                din = DIMS[l]
                dpad = cfg.DPAD if l == 0 else HID  # gather row width
                nh = (din + P - 1) // P  # k-halves
                hselfsrc = hself0 if l == 0 else agin
                lastcols = [min(P, din - a * P) for a in range(nh)]

                # per-layer persistent psum accumulators
                S_ps = [pp.tile([P, HID], F32, tag="Sps", bufs=3,
                                name=f"Sps{l}_{a}") for a in range(nh)]
                sumX_ps = pp.tile([P, HID], F32, tag="Sps", bufs=3,
                                  name=f"sumXps{l}") if nh < 3 else None
                if sumX_ps is None:
                    raise AssertionError("din>256 unsupported")

                g_tiles = {}
                # stream chunk s -> (group key, slot)
                chunk_grp = {}
                for bb in range(NBLK):
                    for gi in range((int(segcnt[bb]) + K - 1) // K):
                        s0g = int(seg0[bb]) + gi * K
                        kk = min(K, int(seg0[bb]) + int(segcnt[bb]) - s0g)
                        for sl in range(kk):
                            chunk_grp[s0g + sl] = (bb, gi, s0g, kk, sl)

                def get_group(s, dpad=dpad, l=l, g_tiles=g_tiles,
                              chunk_grp=chunk_grp):
                    bb, gi, s0g, kk, sl = chunk_grp[s]
                    gkey = (bb, gi)
                    if gkey not in g_tiles:
                        gt_ = plG.tile([P, K * HID], F32, tag="G", name="Gt")
                        if not gathers:
                            g_tiles[gkey] = gt_
                            return g_tiles[gkey], sl
                        if l == 0:
                            src_ap = h0p[BLKOFF[bb]:BLKOFF[bb] + BSZ[bb], :]
                        else:
                            src_ap = h_blk[l - 1][bb][:]
                        nc.gpsimd.dma_gather(
                            out_ap=gt_[:, :kk * dpad]
                                .rearrange("p (k d) -> p k d", d=dpad),
                            in_ap=src_ap,
                            idxs_ap=idx_sb[:, 8 * s0g:8 * (s0g + kk)],
                            num_idxs=P * kk,
                            num_idxs_reg=P * kk,
                            elem_size=dpad,
                            single_packet=False,
                        )
                        g_tiles[gkey] = gt_
                    return g_tiles[gkey], sl

                # ---------- pass 1 ----------
                for t in range(NT):
                    ct = int(c_tb[t].sum())
                    nodes_t = min(P, SHARD - t * P)
                    agg = pp.tile([P, HID], F32, tag="agg", bufs=3, name="agg")
                    gs = plgs.tile([P, HID], F32, tag="gs", name="gs")
                    if nodes_t < P:
                        nc.vector.memset(gs[:, :din], 0.0)
                    nc.sync.dma_start(
                        out=gs[:nodes_t, :din],
                        in_=hselfsrc[t * P:t * P + nodes_t, :din])
                    if ct > 0:
                        M = plM.tile([P, cmax * P], F32, tag="M", name="M")
                        jj0 = 0
                        for bb in range(NBLK):
                            ctb = int(c_tb[t][bb])
                            if ctb == 0:
                                continue
                            cb0 = int(colb0[t][bb])
                            nc.vector.tensor_tensor(
                                out=M[:, jj0 * P:(jj0 + ctb) * P]
                                    .rearrange("p (c f) -> p c f", f=P),
                                in0=iota_sb[:].rearrange("p (o f) -> p o f", o=1)
                                    .to_broadcast([P, ctb, P]),
                                in1=dstloc_sb[:, cb0:cb0 + ctb]
                                    .rearrange("p (c o) -> p c o", o=1)
                                    .to_broadcast([P, ctb, P]),
                                op=OP.is_equal,
                            )
                            jj0 += ctb
                        jj = 0
                        for bb in range(NBLK):
                            for j in range(int(c_tb[t][bb])):
                                s = int(colb0[t][bb]) + j
                                gt_, sl = get_group(s)
                                nc.tensor.matmul(
                                    out=agg[:, :din],
                                    lhsT=M[:, jj * P:(jj + 1) * P],
                                    rhs=gt_[:, sl * dpad:sl * dpad + din],
                                    start=(jj == 0), stop=False,
                                )
                                jj += 1
                    nc.tensor.matmul(
                        out=agg[:, :din], lhsT=twoI_sb[:], rhs=gs[:, :din],
                        start=(ct == 0), stop=True,
                    )
                    X = plX.tile([P, HID], F32, tag="X", name="X")
                    nc.vector.tensor_copy(out=X[:, :din], in_=agg[:, :din])
                    # stats
                    nc.tensor.matmul(out=sumX_ps[0:1, :din], lhsT=ones_sb[:, 0:1],
                                     rhs=X[:, :din], start=(t == 0),
                                     stop=(t == NT - 1))
                    for a in range(nh):
                        ka = lastcols[a]
                        nc.tensor.matmul(
                            out=S_ps[a][:ka, :din], lhsT=X[:, a * P:a * P + ka],
                            rhs=X[:, :din], start=(t == 0), stop=(t == NT - 1))
                    # transpose X -> XT, spill to DRAM
                    xt_ps = pp.tile([P, HID], F32, tag="xt", bufs=2, name="xtps")
                    for a in range(nh):
                        ka = lastcols[a]
                        nc.tensor.transpose(
                            out=xt_ps[:ka, a * P:a * P + P],
                            in_=X[:, a * P:a * P + ka],
                            identity=ident_sb[:],
                        )
                    xt_sb = plxt.tile([P, HID], F32, tag="xtsb", name="xtsb")
                    for a in range(nh):
                        ka = lastcols[a]
                        nc.scalar.copy(out=xt_sb[:ka, a * P:a * P + P],
                                       in_=xt_ps[:ka, a * P:a * P + P])
                    kmax = min(P, din)
                    nc.scalar.dma_start(
                        out=xt_dram[:kmax, t * HID:t * HID + nh * P],
                        in_=xt_sb[:kmax, :nh * P])

                # ---------- per-layer stats & BN folding ----------
                S_sb = []
                for a in range(nh):
                    ka = lastcols[a]
                    st = plst.tile([P, HID], F32, tag=f"Ssb{a}", name=f"Ssb{a}")
                    nc.scalar.copy(out=st[:ka, :din], in_=S_ps[a][:ka, :din])
                    S_sb.append(st)
                sumX_sb = plst.tile([1, HID], F32, tag="sumXsb", name="sumXsb")
                nc.scalar.copy(out=sumX_sb[:, :din], in_=sumX_ps[0:1, :din])
                # sumY2 = colsum(W * (S @ W))   (S symmetric)
                y2s_ps = pp.tile([P, HID], F32, tag="agg", bufs=3, name="y2sps")
                for b in range(nh):
                    kb = lastcols[b]
                    SW_ps = pp.tile([P, HID], F32, tag="xt", bufs=2, name="SWps")
                    for a in range(nh):
                        ka = lastcols[a]
                        nc.tensor.matmul(
                            out=SW_ps[:kb, :],
                            lhsT=S_sb[a][:ka, b * P:b * P + kb],
                            rhs=W_sb[l][a][:ka, :],
                            start=(a == 0), stop=(a == nh - 1))
                    wsw = plst.tile([P, HID], F32, tag="wsw", name="wsw")
                    nc.vector.tensor_mul(out=wsw[:kb, :], in0=W_sb[l][b][:kb, :],
                                         in1=SW_ps[:kb, :])
                    nc.tensor.matmul(out=y2s_ps[0:1, :], lhsT=ones_sb[:kb, 0:1],
                                     rhs=wsw[:kb, :], start=(b == 0),
                                     stop=(b == nh - 1))
                # sumY = sumX @ W : transpose sumX, then matmul
                sxc = plst.tile([P, 2], F32, tag="sxc", name="sxc")
                for a in range(nh):
                    ka = lastcols[a]
                    tp = pp.tile([P, HID], F32, tag="xt", bufs=2, name="tp1")
                    nc.tensor.transpose(out=tp[:ka, 0:1],
                                        in_=sumX_sb[0:1, a * P:a * P + ka],
                                        identity=ident_sb[0:1, 0:1])
                    nc.scalar.copy(out=sxc[:ka, a:a + 1], in_=tp[:ka, 0:1])
                sy_ps = pp.tile([P, HID], F32, tag="xt", bufs=2, name="syps")
                for a in range(nh):
                    ka = lastcols[a]
                    nc.tensor.matmul(out=sy_ps[0:1, :], lhsT=sxc[:ka, a:a + 1],
                                     rhs=W_sb[l][a][:ka, :], start=(a == 0),
                                     stop=(a == nh - 1))
                arp = plst.tile([1, 2 * HID], F32, tag="arp", name="arp")
                nc.scalar.copy(out=arp[0:1, :HID], in_=sy_ps[0:1, :])
                nc.scalar.copy(out=arp[0:1, HID:], in_=y2s_ps[0:1, :])
                nc.sync.dma_start(out=ar_in[l][:], in_=arp[:])
                if coll:
                    nc.gpsimd.collective_compute(
                        "AllReduce", OP.add,
                        replica_groups=[list(range(NC_))],
                        ins=[ar_in[l][:]], outs=[ar_out[l][:]],
                    )
                else:
                    nc.sync.dma_start(out=ar_out[l][:], in_=ar_in[l][:])
                arr = plst.tile([1, 2 * HID], F32, tag="arr", name="arr")
                nc.sync.dma_start(out=arr[:], in_=ar_out[l][:])
                mean = plst.tile([1, HID], F32, tag="mean", name="mean")
                nc.scalar.mul(out=mean[:], in_=arr[0:1, :HID], mul=1.0 / N)
                ex2 = plst.tile([1, HID], F32, tag="stmp0", name="ex2")
                nc.scalar.mul(out=ex2[:], in_=arr[0:1, HID:], mul=1.0 / N)
                msq = plst.tile([1, HID], F32, tag="stmp1", name="msq")
                nc.scalar.activation(out=msq[:], in_=mean[:], func=AF.Square)
                var = plst.tile([1, HID], F32, tag="stmp2", name="var")
                nc.vector.tensor_sub(out=var[:], in0=ex2[:], in1=msq[:])
                vep = plst.tile([1, HID], F32, tag="stmp0", name="vep")
                nc.vector.tensor_scalar_add(out=vep[:], in0=var[:], scalar1=EPS_BN)
                rv = plst.tile([1, HID], F32, tag="stmp1", name="rv")
                nc.vector.reciprocal(out=rv[:], in_=vep[:])
                rs = plst.tile([1, HID], F32, tag="stmp2", name="rs")
                nc.scalar.activation(out=rs[:], in_=rv[:], func=AF.Sqrt)
                A_sb = plst.tile([1, HID], F32, tag="A_sb", name="A_sb")
                nc.vector.tensor_mul(out=A_sb[:], in0=rs[:], in1=bnp_sb[0:1, l * HID:(l + 1) * HID])
                mA = plst.tile([1, HID], F32, tag="stmp0", name="mA")
                nc.vector.tensor_mul(out=mA[:], in0=mean[:], in1=A_sb[:])
                B_sb = plst.tile([1, HID], F32, tag="B_sb", name="B_sb")
                nc.vector.tensor_sub(out=B_sb[:], in0=bnp_sb[0:1, (3 + l) * HID:(4 + l) * HID],
                                     in1=mA[:])
                arep_ps = pp.tile([P, HID], F32, tag="xt", bufs=2, name="arepps")
                nc.tensor.matmul(out=arep_ps[:], lhsT=ones_sb[0:1, :],
                                 rhs=A_sb[:], start=True, stop=True)
                Weff = []
                for a in range(nh):
                    ka = lastcols[a]
                    we = plweff.tile([P, HID], F32, tag="weff", name="weff")
                    nc.vector.tensor_mul(out=we[:ka, :], in0=W_sb[l][a][:ka, :],
                                         in1=arep_ps[:ka, :])
                    Weff.append(we)

                # ---------- pass 2 ----------
                if l == 2:
                    xread_ps = pp.tile([P, HID], F32, tag="Sps", bufs=3,
                                       name="xreadps")
                for t in range(NT):
                    nodes_t = min(P, SHARD - t * P)
                    kmax = min(P, din)
                    xt_ld = plxt.tile([P, HID], F32, tag="xtld", name="xtld")
                    nc.scalar.dma_start(out=xt_ld[:kmax, :nh * P],
                                        in_=xt_dram[:kmax, t * HID:t * HID + nh * P])
                    y2 = pp.tile([P, max(GQ, HID)], F32, tag="xt", bufs=2,
                                 name="y2")
                    for a in range(nh):
                        ka = lastcols[a]
                        nc.tensor.matmul(
                            out=y2[:, :HID],
                            lhsT=xt_ld[:ka, a * P:a * P + P],
                            rhs=Weff[a][:ka, :], start=(a == 0), stop=False)
                    nc.tensor.matmul(out=y2[:, :HID], lhsT=ones_sb[0:1, :],
                                     rhs=B_sb[:], start=False, stop=True)
                    hnew = plh.tile([P, HID], F32, tag="hnew", name="hnew")
                    nc.scalar.activation(out=hnew[:], in_=y2[:, :HID], func=AF.Relu)
                    if l < 2:
                        nc.sync.dma_start(
                            out=agin[t * P:t * P + nodes_t, :],
                            in_=hnew[:nodes_t, :])
                        qends = np.cumsum(TQ) - 1
                        if t in qends:
                            qq = int(np.where(qends == t)[0][0])
                            if coll:
                                nc.gpsimd.collective_compute(
                                    "AllGather", OP.bypass,
                                    replica_groups=[list(range(NC_))],
                                    ins=[agin[QOFF[qq]:QOFF[qq] + QB[qq], :]],
                                    outs=[h_blk[l][qq][:]],
                                )
                            else:
                                nc.sync.dma_start(
                                    out=h_blk[l][qq][0:QB[qq], :],
                                    in_=agin[QOFF[qq]:QOFF[qq] + QB[qq], :])
                    else:
                        R = plM.tile([P, P], F32, tag="R", name="R")
                        nc.vector.tensor_tensor(
                            out=R[:], in0=iota_sb[:],
                            in1=gidloc_sb[:, t:t + 1].to_broadcast([P, P]),
                            op=OP.is_equal)
                        nc.tensor.matmul(out=xread_ps[:], lhsT=R[:], rhs=hnew[:],
                                         start=(t == 0), stop=(t == NT - 1))

            # ---------------- readout exchange ----------------
            xp_sb = plst.tile([P, HID], F32, tag="xp_sb", name="xp_sb")
            nc.scalar.copy(out=xp_sb[:], in_=xread_ps[:])
            nc.sync.dma_start(out=xag_in[:], in_=xp_sb[:])
            if coll:
                nc.gpsimd.collective_compute(
                    "AllGather", OP.bypass,
                    replica_groups=[list(range(NC_))],
                    ins=[xag_in[:]], outs=[xag_out[:]],
                )
            else:
                nc.sync.dma_start(out=xag_out[0:P, :], in_=xag_in[:])
            xg_sb = plst.tile([P, NC_ * HID], F32, tag="xg_sb", name="xg_sb")
            nc.sync.dma_start(
                out=xg_sb[:].rearrange("p (c f) -> p c f", f=HID),
                in_=xag_out[:].rearrange("(c p) f -> p c f", p=P))
            xfull_sb = []
            for gt in range(GT):
                xf_ps = pp.tile([P, HID], F32, tag="agg", bufs=3, name="xfps")
                for c in range(NC_):
                    gsh = plM.tile([P, 1], F32, tag="gsh", name="gsh")
                    nc.vector.tensor_scalar(
                        out=gsh[:], in0=growid_sb[:, c:c + 1],
                        scalar1=float(P * gt), scalar2=None, op0=OP.subtract)
                    R2 = plM.tile([P, P], F32, tag="R", name="R2")
                    nc.vector.tensor_tensor(
                        out=R2[:], in0=iota_sb[:],
                        in1=gsh[:].to_broadcast([P, P]), op=OP.is_equal)
                    nc.tensor.matmul(out=xf_ps[:],
                                     lhsT=R2[:],
                                     rhs=xg_sb[:, c * HID:(c + 1) * HID],
                                     start=(c == 0), stop=(c == NC_ - 1))
                xf = plst.tile([P, HID], F32, tag=f"xfull{gt}", name=f"xfull{gt}")
                nc.scalar.copy(out=xf[:], in_=xf_ps[:])
                xfull_sb.append(xf)
            # transpose -> xT [HID(2 tiles), GQ]
            xT_sb = [plst.tile([P, GQ], F32, tag=f"xT{h}", name=f"xT{h}")
                     for h in range(HID // P)]
            for h in range(HID // P):
                for gt in range(GT):
                    tp2 = pp.tile([P, HID], F32, tag="xt", bufs=2, name="tp2")
                    nc.tensor.transpose(out=tp2[:, :P],
                                        in_=xfull_sb[gt][:, h * P:(h + 1) * P],
                                        identity=ident_sb[:])
                    nc.scalar.copy(out=xT_sb[h][:, gt * P:(gt + 1) * P],
                                   in_=tp2[:, :P])

            def bn_relu_T(y_ps, g_ap, b_ap, nb, tagsfx):
                """BN(train)+relu on feature-major psum tile [128, nb]."""
                s1 = plst.tile([P, 1], F32, tag="s1", name="s1")
                nc.vector.tensor_reduce(out=s1[:], in_=y_ps[:, :nb],
                                        axis=AX.X, op=OP.add)
                sq = plst.tile([P, GQ], F32, tag="sq", name="sq")
                s2 = plst.tile([P, 1], F32, tag="s2", name="s2")
                nc.scalar.activation(out=sq[:, :nb], in_=y_ps[:, :nb],
                                     func=AF.Square, accum_out=s2[:])
                mn = plst.tile([P, 1], F32, tag="mn", name="mn")
                nc.scalar.mul(out=mn[:], in_=s1[:], mul=1.0 / nb)
                e2 = plst.tile([P, 1], F32, tag="e2", name="e2")
                nc.scalar.mul(out=e2[:], in_=s2[:], mul=1.0 / nb)
                ms = plst.tile([P, 1], F32, tag="ms", name="ms")
                nc.scalar.activation(out=ms[:], in_=mn[:], func=AF.Square)
                vr = plst.tile([P, 1], F32, tag="vr", name="vr")
                nc.vector.tensor_sub(out=vr[:], in0=e2[:], in1=ms[:])
                ve = plst.tile([P, 1], F32, tag="ve", name="ve")
                nc.vector.tensor_scalar_add(out=ve[:], in0=vr[:], scalar1=EPS_BN)
                rv2 = plst.tile([P, 1], F32, tag="rv2", name="rv2")
                nc.vector.reciprocal(out=rv2[:], in_=ve[:])
                rs2 = plst.tile([P, 1], F32, tag="rs2", name="rs2")
                nc.scalar.activation(out=rs2[:], in_=rv2[:], func=AF.Sqrt)
                Am = plst.tile([P, 1], F32, tag="Am", name="Am")
                nc.vector.tensor_mul(out=Am[:], in0=rs2[:], in1=g_ap)
                mAm = plst.tile([P, 1], F32, tag="mAm", name="mAm")
                nc.vector.tensor_mul(out=mAm[:], in0=mn[:], in1=Am[:])
                Bm = plst.tile([P, 1], F32, tag="Bm", name="Bm")
                nc.vector.tensor_sub(out=Bm[:], in0=b_ap, in1=mAm[:])
                yo = plst.tile([P, GQ], F32, tag="yo", bufs=6, name=f"yo{tagsfx}")
                if nb < GQ:
                    nc.vector.memset(yo[:], 0.0)
                nc.scalar.activation(out=yo[:, :nb], in_=y_ps[:, :nb], func=AF.Relu,
                                     scale=Am[:], bias=Bm[:])
                return yo

            # FC1 -> BN -> relu (feature-major), then hidden MLP, then FC2
            y1t = []
            for m in range(cfg.MLP0 // P):
                y1_ps = pp.tile([P, max(GQ, HID)], F32, tag="xt", bufs=2,
                                name="y1ps")
                for k in range(HID // P):
                    nc.tensor.matmul(out=y1_ps[:, :GQ],
                                     lhsT=fc1_sb[k][:, m * P:(m + 1) * P],
                                     rhs=xT_sb[k][:], start=(k == 0),
                                     stop=(k == HID // P - 1))
                y1t.append(bn_relu_T(y1_ps, bn1g_sb[:, m:m + 1],
                                     bn1b_sb[:, m:m + 1], cfg.G, f"a{m}"))
            y2t = []
            for m in range(cfg.MLP1 // P):
                y2_ps = pp.tile([P, max(GQ, HID)], F32, tag="xt", bufs=2,
                                name="y2ps")
                for k in range(cfg.MLP0 // P):
                    nc.tensor.matmul(out=y2_ps[:, :GQ],
                                     lhsT=mlpW_sb[k][:, m * P:(m + 1) * P],
                                     rhs=y1t[k][:], start=(k == 0),
                                     stop=(k == cfg.MLP0 // P - 1))
                y2t.append(bn_relu_T(y2_ps, mbng_sb[:, m:m + 1],
                                     mbnb_sb[:, m:m + 1], cfg.G, f"b{m}"))
            for gt in range(GT):
                ng = min(P, cfg.G - gt * P)
                lg_ps = pp.tile([P, HID], F32, tag="agg", bufs=3, name="lgps")
                for k in range(cfg.MLP1 // P):
                    nc.tensor.matmul(out=lg_ps[:, :cfg.NCLS],
                                     lhsT=y2t[k][:, gt * P:gt * P + P],
                                     rhs=fc2_sb[k][:], start=(k == 0), stop=False)
                nc.tensor.matmul(out=lg_ps[:, :cfg.NCLS], lhsT=ones_sb[0:1, :],
                                 rhs=fc2b_sb[:], start=False, stop=True)
                ot = plh.tile([P, cfg.NCLS], F32, tag="ot", name="ot")
                nc.scalar.activation(out=ot[:ng, :], in_=lg_ps[:ng, :cfg.NCLS],
                                     func=AF.Sigmoid)
                nc.sync.dma_start(out=out_d[gt * P:gt * P + ng, :], in_=ot[:ng, :])


def _pack_inputs(inputs, cfg, meta):
    """Build per-core in_maps."""
    NC_ = cfg.NC
    h = np.ascontiguousarray(np.asarray(inputs["h"], np.float32))
    h0p = np.zeros((cfg.N, cfg.DPAD), np.float32)
    perm = np.empty(cfg.N, np.int64)
    pos = 0
    for b in range(cfg.NBLK):
        for c in range(NC_):
            s0 = c * cfg.SHARD + cfg.QOFF[b]
            perm[pos:pos + cfg.QB[b]] = np.arange(s0, s0 + cfg.QB[b])
            pos += cfg.QB[b]
    h0p[:, :cfg.IN_FEATS] = h[perm]
    iota = np.ascontiguousarray(
        np.tile(np.arange(P, dtype=np.float32)[None, :], (P, 1)))
    twoI = np.ascontiguousarray(2.0 * np.eye(P, dtype=np.float32))
    ident = np.ascontiguousarray(np.eye(P, dtype=np.float32))
    ones = np.ones((P, P), np.float32)
    Ws = [np.ascontiguousarray(np.asarray(w, np.float32)) for w in inputs["gcn_Ws"]]
    bnp = np.ascontiguousarray(np.concatenate(
        [np.asarray(x, np.float32) for x in inputs["bn_gs"]] +
        [np.asarray(x, np.float32) for x in inputs["bn_bs"]])[None, :])
    fc1W = np.ascontiguousarray(np.asarray(inputs["fc1_W"], np.float32))
    bn1g = np.ascontiguousarray(
        np.asarray(inputs["bn1_g"], np.float32).reshape(-1, P).T)
    bn1b = np.ascontiguousarray(
        np.asarray(inputs["bn1_b"], np.float32).reshape(-1, P).T)
    mlpW = np.ascontiguousarray(np.asarray(inputs["mlp_Ws"][0], np.float32))
    mbng = np.ascontiguousarray(
        np.asarray(inputs["mlp_bn_gs"][0], np.float32).reshape(-1, P).T)
    mbnb = np.ascontiguousarray(
        np.asarray(inputs["mlp_bn_bs"][0], np.float32).reshape(-1, P).T)
    fc2W = np.ascontiguousarray(
        np.asarray(inputs["fc2_W"], np.float32)[:, -cfg.NCLS:])
    fc2b = np.ascontiguousarray(
        np.asarray(inputs["fc2_b"], np.float32)[None, -cfg.NCLS:])
    in_maps = []
    for c in range(NC_):
        in_maps.append(dict(
            h0p=h0p,
            hself0=np.ascontiguousarray(h[c * cfg.SHARD:(c + 1) * cfg.SHARD]),
            idx16=np.ascontiguousarray(meta["idx16"][c]),
            dstloc=np.ascontiguousarray(meta["dstloc"][c]),
            gidloc=np.ascontiguousarray(meta["gidloc"][c]),
            growid=np.ascontiguousarray(meta["growid"]),
            iota=iota, twoI=twoI, ident=ident, ones=ones,
            W0=Ws[0], W1=Ws[1], W2=Ws[2], bnp=bnp,
            fc1W=fc1W, bn1gT=bn1g, bn1bT=bn1b,
            mlpW=mlpW, mbngT=mbng, mbnbT=mbnb,
            fc2Wr=fc2W, fc2br=fc2b,
        ))
    return in_maps


def make_nc(cfg, meta, coll=True, gathers=True, num_devices=None):
    nc = bacc.Bacc("TRN2", target_bir_lowering=False, debug=False,
                   enable_asserts=False,
                   num_devices=num_devices or (cfg.NC if coll else 1))
    _build(nc, cfg, meta, coll=coll, gathers=gathers)
    nc.compile()
    return nc


def build_and_run(inputs, cfg, **run_kwargs):
    meta = _plan(inputs["src"], inputs["dst"], inputs["graph_id"], cfg)
    nc = make_nc(cfg, meta)
    in_maps = _pack_inputs(inputs, cfg, meta)
    res = run_bass_kernel_spmd(nc, in_maps, core_ids=list(range(cfg.NC)),
                               **run_kwargs)
    return res


def kernel(**inputs):
    cfg = Cfg()
    res = build_and_run(inputs, cfg)
    return np.asarray(res.results[0]["out"], np.float32)


# revision 24
# speedup vs baseline: 1.1026x; 1.0605x over previous
"""GIN message-passing classifier on 8 Trainium2 NeuronCores.

Strategy (graph/node partition, data parallel):
  - Nodes are split into 8 equal contiguous shards (12500 nodes/core); each
    core owns the edges whose *destination* lands in its shard.
  - Host pre-sorts edges by dst, groups them per 128-node tile, pads each
    tile's edge list to a multiple of 128 ("chunks").  Chunk counts are taken
    as the max over cores so all 8 cores run one identical program (SPMD).
  - On device, chunks of 128 edges are gathered with batched indirect DMA
    (h[src] rows) and scatter-added into the tile's PSUM accumulator with a
    one-hot selector matmul built on the fly by the vector engine (is_equal
    against an iota row).  The GIN self term (2*h) is one extra matmul with a
    constant 2*I selector on contiguously-loaded own rows.
  - BatchNorm batch statistics come from per-core partials (sum(X) and the
    second moment X^T X pushed through W analytically), combined with a tiny
    [2,256] AllReduce; scale/shift are folded into W on device, so the
    per-node epilogue is matmul + fused relu.
  - h_new is exchanged between layers with an AllGather (rows = node shards).
  - Readout (segment-sum per graph) uses the same one-hot-selector matmul
    into per-core local graph slots, a small AllGather, and a host-planned
    slot->graph selector reduce; the graph-level MLP is replicated.
"""

import numpy as np

import concourse.bass as bass
import concourse.mybir as mybir
import concourse.tile as tile
from concourse import bacc
from concourse.bass_utils import run_bass_kernel_spmd

P = 128


def _T(tc, *args, **kw):
    t, _free = tc.tile(*args, **kw)
    return t


F32 = mybir.dt.float32
I32 = mybir.dt.int32
AF = mybir.ActivationFunctionType
OP = mybir.AluOpType
AX = mybir.AxisListType
EPS_BN = 1e-5


class Cfg:
    def __init__(self, N=100000, E=1000000, G=512, IN_FEATS=78, HID=256,
                 MLP0=512, MLP1=256, NCLS=204, NC=8, K_GATHER=8):
        self.N, self.E, self.G = N, E, G
        self.IN_FEATS, self.HID = IN_FEATS, HID
        self.MLP0, self.MLP1, self.NCLS = MLP0, MLP1, NCLS
        self.NC, self.K = NC, K_GATHER
        assert N % NC == 0
        self.SHARD = N // NC
        self.NT = (self.SHARD + P - 1) // P
        self.GT = (G + P - 1) // P
        self.NBLK = 4
        # block b = all cores' quarter-b of their shard; quarters are
        # tile-aligned so pipelined AllGathers unblock gather segments.
        tq = (self.NT + 3) // 4           # tiles per quarter (last smaller)
        self.TQ = [tq, tq, tq, self.NT - 3 * tq]
        qb = [min(t * P, self.SHARD) for t in np.cumsum([0] + self.TQ)]
        self.QOFF = qb[:4]                # row offset of quarter q in shard
        self.QB = [qb[i + 1] - qb[i] for i in range(4)]  # rows per quarter
        self.BSZ = [self.NC * q for q in self.QB]        # rows per block
        self.BLKOFF = [0]
        for b in range(3):
            self.BLKOFF.append(self.BLKOFF[-1] + self.BSZ[b])
        assert max(self.BSZ) < 32768, "dma_gather int16 index range"
        self.DPAD = 128  # layer-0 gather row padded to 128 f32 (512B)


def _plan(src, dst, graph_id, cfg):
    """Host-side edge bucketing. Returns per-core data + shared metadata.

    Edges are bucketed by (dst-tile, src-block); the chunk stream is laid out
    block-major (all tiles' block-0 chunks, then block-1, ...) so each
    dma_gather instruction covers one contiguous same-block run of chunks.
    """
    NC, SHARD, NT = cfg.NC, cfg.SHARD, cfg.NT
    NBLK = cfg.NBLK
    qoff = np.asarray(cfg.QOFF + [SHARD], np.int64)
    qb = np.asarray(cfg.QB, np.int64)
    src = np.asarray(src).astype(np.int64).ravel()
    dst = np.asarray(dst).astype(np.int64).ravel()
    gid = np.asarray(graph_id).astype(np.int64).ravel()

    core = dst // SHARD
    rem = dst % SHARD
    t = rem // P
    loc = rem % P
    score = src // SHARD
    soff = src % SHARD
    b = np.searchsorted(qoff, soff, side="right") - 1
    key = (core * NT + t) * NBLK + b
    order = np.argsort(key, kind="stable")
    key = key[order]
    srcl = (score * qb[b] + (soff - qoff[b]))[order]
    assert srcl.max() < 32768
    loc = loc[order]
    cnt = np.bincount(key, minlength=NC * NT * NBLK).reshape(NC, NT, NBLK)
    c_tb = ((cnt + P - 1) // P).max(axis=0).astype(np.int64)  # [NT, NBLK]
    segcnt = c_tb.sum(axis=0)                                  # [NBLK]
    seg0 = np.zeros(NBLK, np.int64)
    seg0[1:] = np.cumsum(segcnt)[:-1]
    # chunk-stream start of each (t, b) run
    colb0 = np.zeros((NT, NBLK), np.int64)
    for bb in range(NBLK):
        colb0[0, bb] = seg0[bb]
        colb0[1:, bb] = seg0[bb] + np.cumsum(c_tb[:-1, bb])
    C = max(int(c_tb.sum()), 1)

    starts = np.zeros(NC * NT * NBLK + 1, np.int64)
    starts[1:] = np.cumsum(cnt.ravel())
    rank = np.arange(len(key)) - starts[key]
    ch = rank // P
    p = rank % P
    tt = (key // NBLK) % NT
    bb_ = key % NBLK
    cc = key // (NT * NBLK)
    s = colb0[tt, bb_] + ch  # stream chunk index
    g = s * P + p            # global slot position

    idx16 = np.zeros((NC, C * P), np.int16)
    dstloc = np.full((NC, P, C), -1.0, np.float32)
    idx16[cc, g] = srcl.astype(np.int16)
    dstloc[cc, p, s] = loc.astype(np.float32)
    # wrapped layout: position g -> [g % 16, g // 16], replicated on 8 Q7 cores
    idx16w = np.zeros((NC, P, (C * P) // 16), np.int16)
    for c in range(NC):
        w = idx16[c].reshape((C * P) // 16, 16).T  # [16, 8C]
        idx16w[c] = np.tile(w, (8, 1))

    gidloc = np.full((NC, P, NT), -1.0, np.float32)
    growid = np.full((P, NC), -1.0, np.float32)
    for c in range(NC):
        gg = gid[c * SHARD:(c + 1) * SHARD]
        gb, gm = int(gg[0]), int(gg[-1])
        assert gm - gb < P, "graph span exceeds 128 per core"
        arr = np.full(NT * P, -1.0, np.float32)
        arr[:SHARD] = (gg - gb).astype(np.float32)
        gidloc[c] = arr.reshape(NT, P).T
        jj = np.arange(P)
        sel = (gb + jj) <= gm
        growid[sel, c] = (gb + jj[sel]).astype(np.float32)

    return dict(c_tb=c_tb, C=C, colb0=colb0, seg0=seg0, segcnt=segcnt,
                idx16=idx16w, dstloc=dstloc, gidloc=gidloc, growid=growid)


def _build(nc, cfg, meta, coll=True, gathers=True):
    """Trace the full Bass/Tile program (shared by all 8 cores)."""
    NT, C, K = cfg.NT, meta["C"], cfg.K
    c_tb, colb0 = meta["c_tb"], meta["colb0"]
    seg0, segcnt = meta["seg0"], meta["segcnt"]
    NBLK = cfg.NBLK
    BSZ, BLKOFF, QOFF, QB, TQ = cfg.BSZ, cfg.BLKOFF, cfg.QOFF, cfg.QB, cfg.TQ
    HID = cfg.HID
    DIMS = [cfg.IN_FEATS, HID, HID]
    N, SHARD, NC_ = cfg.N, cfg.SHARD, cfg.NC
    GT = cfg.GT
    GQ = GT * P

    # ---------------- DRAM I/O ----------------
    h0p = nc.dram_tensor("h0p", [N, cfg.DPAD], F32, kind="ExternalInput").ap()
    hself0 = nc.dram_tensor("hself0", [SHARD, cfg.IN_FEATS], F32,
                            kind="ExternalInput").ap()
    idx_d = nc.dram_tensor("idx16", [P, (C * P) // 16], mybir.dt.int16,
                           kind="ExternalInput").ap()
    dstloc_d = nc.dram_tensor("dstloc", [P, C], F32, kind="ExternalInput").ap()
    gidloc_d = nc.dram_tensor("gidloc", [P, NT], F32, kind="ExternalInput").ap()
    growid_d = nc.dram_tensor("growid", [P, NC_], F32, kind="ExternalInput").ap()
    iota_d = nc.dram_tensor("iota", [P, P], F32, kind="ExternalInput").ap()
    twoI_d = nc.dram_tensor("twoI", [P, P], F32, kind="ExternalInput").ap()
    ident_d = nc.dram_tensor("ident", [P, P], F32, kind="ExternalInput").ap()
    ones_d = nc.dram_tensor("ones", [P, P], F32, kind="ExternalInput").ap()
    W_d = [nc.dram_tensor(f"W{i}", [DIMS[i], HID], F32, kind="ExternalInput").ap()
           for i in range(3)]
    bnp_d = nc.dram_tensor("bnp", [1, 6 * HID], F32,
                           kind="ExternalInput").ap()
    fc1_d = nc.dram_tensor("fc1W", [HID, cfg.MLP0], F32, kind="ExternalInput").ap()
    bn1g_d = nc.dram_tensor("bn1gT", [P, cfg.MLP0 // P], F32,
                            kind="ExternalInput").ap()
    bn1b_d = nc.dram_tensor("bn1bT", [P, cfg.MLP0 // P], F32,
                            kind="ExternalInput").ap()
    mlpW_d = nc.dram_tensor("mlpW", [cfg.MLP0, cfg.MLP1], F32,
                            kind="ExternalInput").ap()
    mbng_d = nc.dram_tensor("mbngT", [P, cfg.MLP1 // P], F32,
                            kind="ExternalInput").ap()
    mbnb_d = nc.dram_tensor("mbnbT", [P, cfg.MLP1 // P], F32,
                            kind="ExternalInput").ap()
    fc2_d = nc.dram_tensor("fc2Wr", [cfg.MLP1, cfg.NCLS], F32,
                           kind="ExternalInput").ap()
    fc2b_d = nc.dram_tensor("fc2br", [1, cfg.NCLS], F32, kind="ExternalInput").ap()
    out_d = nc.dram_tensor("out", [cfg.G, cfg.NCLS], F32, kind="ExternalOutput").ap()

    with tile.TileContext(nc) as tc, \
            tc.tile_pool(name="plD", bufs=1, space="DRAM") as plD, \
            tc.tile_pool(name="plC", bufs=1) as plC:
        def DT(name, shape, addr_space="Local"):
            return plD.tile(shape, F32, tag=name, name=name,
                            addr_space=addr_space)

        def CT(name, shape, dtype=F32, src_ap=None):
            t = plC.tile(shape, dtype, tag=name, name=name)
            if src_ap is not None:
                nc.sync.dma_start(out=t[:], in_=src_ap)
            return t

        # shared DRAM scratch: per layer, 4 block buffers (block b = all
        # cores' quarter-b rows, rank-major within the block)
        h_blk = [[DT(f"hx{i}b{b}", [BSZ[b], HID], addr_space="Shared")
                  for b in range(NBLK)] for i in range(2)]
        agin = DT("agin", [SHARD, HID])
        ar_in = [DT(f"arin{i}", [1, 2 * HID]) for i in range(3)]
        ar_out = [DT(f"arout{i}", [1, 2 * HID], addr_space="Shared")
                  for i in range(3)]
        xag_in = DT("xagin", [P, HID])
        xag_out = DT("xagout", [NC_ * P, HID], addr_space="Shared")
        xt_dram = DT("xt_dram", [P, NT * HID])

        # ---------------- persistent SBUF ----------------
        idx_sb = CT("idx_sb", [P, (C * P) // 16], mybir.dt.int16, idx_d)
        dstloc_sb = CT("dstloc_sb", [P, C], F32, dstloc_d)
        gidloc_sb = CT("gidloc_sb", [P, NT], F32, gidloc_d)
        growid_sb = CT("growid_sb", [P, NC_], F32, growid_d)
        iota_sb = CT("iota_sb", [P, P], F32, iota_d)
        twoI_sb = CT("twoI_sb", [P, P], F32, twoI_d)
        ident_sb = CT("ident_sb", [P, P], F32, ident_d)
        ones_sb = CT("ones_sb", [P, P], F32, ones_d)

        W_sb = []  # per layer: list of [128, HID] k-half tiles
        for l in range(3):
            din = DIMS[l]
            halves = []
            for a in range((din + P - 1) // P):
                ka = min(P, din - a * P)
                wt = CT(f"Wsb{l}_{a}", [P, HID])
                nc.sync.dma_start(out=wt[:ka, :], in_=W_d[l][a * P:a * P + ka, :])
                halves.append(wt)
            W_sb.append(halves)
        bnp_sb = CT("bnp_sb", [1, 6 * HID], F32, bnp_d)

        fc1_sb = [CT(f"fc1sb{a}", [P, cfg.MLP0]) for a in range(HID // P)]
        for a in range(HID // P):
            nc.sync.dma_start(out=fc1_sb[a][:], in_=fc1_d[a * P:(a + 1) * P, :])
        bn1g_sb = CT("bn1g_sb", [P, cfg.MLP0 // P], F32, bn1g_d)
        bn1b_sb = CT("bn1b_sb", [P, cfg.MLP0 // P], F32, bn1b_d)
        mlpW_sb = [CT(f"mlpWsb{a}", [P, cfg.MLP1]) for a in range(cfg.MLP0 // P)]
        for a in range(cfg.MLP0 // P):
            nc.sync.dma_start(out=mlpW_sb[a][:], in_=mlpW_d[a * P:(a + 1) * P, :])
        mbng_sb = CT("mbng_sb", [P, cfg.MLP1 // P], F32, mbng_d)
        mbnb_sb = CT("mbnb_sb", [P, cfg.MLP1 // P], F32, mbnb_d)
        fc2_sb = [CT(f"fc2sb{a}", [P, cfg.NCLS]) for a in range(cfg.MLP1 // P)]
        for a in range(cfg.MLP1 // P):
            nc.sync.dma_start(out=fc2_sb[a][:], in_=fc2_d[a * P:(a + 1) * P, :])
        fc2b_sb = CT("fc2b_sb", [1, cfg.NCLS], F32, fc2b_d)

        cmax = max(int(c_tb.sum(axis=1).max()), 1)

        with (
            tc.tile_pool(name="plG", bufs=7) as plG,
            tc.tile_pool(name="plgs", bufs=3) as plgs,
            tc.tile_pool(name="plM", bufs=2) as plM,
            tc.tile_pool(name="plX", bufs=3) as plX,
            tc.tile_pool(name="plh", bufs=3) as plh,
            tc.tile_pool(name="plxt", bufs=3) as plxt,
            tc.tile_pool(name="plst", bufs=1) as plst,
            tc.tile_pool(name="plweff", bufs=2) as plweff,
            tc.tile_pool(name="pp", bufs=1, space="PSUM") as pp,
        ):
            for l in range(3):
                din = DIMS[l]
                dpad = cfg.DPAD if l == 0 else HID  # gather row width
                nh = (din + P - 1) // P  # k-halves
                hselfsrc = hself0 if l == 0 else agin
                lastcols = [min(P, din - a * P) for a in range(nh)]

                # per-layer persistent psum accumulators
                S_ps = [pp.tile([P, HID], F32, tag="Sps", bufs=3,
                                name=f"Sps{l}_{a}") for a in range(nh)]
                sumX_ps = pp.tile([P, HID], F32, tag="Sps", bufs=3,
                                  name=f"sumXps{l}") if nh < 3 else None
                if sumX_ps is None:
                    raise AssertionError("din>256 unsupported")

                g_tiles = {}
                # stream chunk s -> (group key, slot)
                chunk_grp = {}
                for bb in range(NBLK):
                    for gi in range((int(segcnt[bb]) + K - 1) // K):
                        s0g = int(seg0[bb]) + gi * K
                        kk = min(K, int(seg0[bb]) + int(segcnt[bb]) - s0g)
                        for sl in range(kk):
                            chunk_grp[s0g + sl] = (bb, gi, s0g, kk, sl)

                def get_group(s, dpad=dpad, l=l, g_tiles=g_tiles,
                              chunk_grp=chunk_grp):
                    bb, gi, s0g, kk, sl = chunk_grp[s]
                    gkey = (bb, gi)
                    if gkey not in g_tiles:
                        gt_ = plG.tile([P, K * HID], F32, tag="G", name="Gt")
                        if not gathers:
                            g_tiles[gkey] = gt_
                            return g_tiles[gkey], sl
                        if l == 0:
                            src_ap = h0p[BLKOFF[bb]:BLKOFF[bb] + BSZ[bb], :]
                        else:
                            src_ap = h_blk[l - 1][bb][:]
                        nc.gpsimd.dma_gather(
                            out_ap=gt_[:, :kk * dpad]
                                .rearrange("p (k d) -> p k d", d=dpad),
                            in_ap=src_ap,
                            idxs_ap=idx_sb[:, 8 * s0g:8 * (s0g + kk)],
                            num_idxs=P * kk,
                            num_idxs_reg=P * kk,
                            elem_size=dpad,
                            single_packet=False,
                        )
                        g_tiles[gkey] = gt_
                    return g_tiles[gkey], sl

                # ---------- pass 1 ----------
                for t in range(NT):
                    ct = int(c_tb[t].sum())
                    nodes_t = min(P, SHARD - t * P)
                    agg = pp.tile([P, HID], F32, tag="agg", bufs=3, name="agg")
                    gs = plgs.tile([P, HID], F32, tag="gs", name="gs")
                    if nodes_t < P:
                        nc.vector.memset(gs[:, :din], 0.0)
                    nc.sync.dma_start(
                        out=gs[:nodes_t, :din],
                        in_=hselfsrc[t * P:t * P + nodes_t, :din])
                    if ct > 0:
                        M = plM.tile([P, cmax * P], F32, tag="M", name="M")
                        jj0 = 0
                        for bb in range(NBLK):
                            ctb = int(c_tb[t][bb])
                            if ctb == 0:
                                continue
                            cb0 = int(colb0[t][bb])
                            nc.vector.tensor_tensor(
                                out=M[:, jj0 * P:(jj0 + ctb) * P]
                                    .rearrange("p (c f) -> p c f", f=P),
                                in0=iota_sb[:].rearrange("p (o f) -> p o f", o=1)
                                    .to_broadcast([P, ctb, P]),
                                in1=dstloc_sb[:, cb0:cb0 + ctb]
                                    .rearrange("p (c o) -> p c o", o=1)
                                    .to_broadcast([P, ctb, P]),
                                op=OP.is_equal,
                            )
                            jj0 += ctb
                        jj = 0
                        for bb in range(NBLK):
                            for j in range(int(c_tb[t][bb])):
                                s = int(colb0[t][bb]) + j
                                gt_, sl = get_group(s)
                                nc.tensor.matmul(
                                    out=agg[:, :din],
                                    lhsT=M[:, jj * P:(jj + 1) * P],
                                    rhs=gt_[:, sl * dpad:sl * dpad + din],
                                    start=(jj == 0), stop=False,
                                )
                                jj += 1
                    nc.tensor.matmul(
                        out=agg[:, :din], lhsT=twoI_sb[:], rhs=gs[:, :din],
                        start=(ct == 0), stop=True,
                    )
                    X = plX.tile([P, HID], F32, tag="X", name="X")
                    nc.vector.tensor_copy(out=X[:, :din], in_=agg[:, :din])
                    # stats
                    nc.tensor.matmul(out=sumX_ps[0:1, :din], lhsT=ones_sb[:, 0:1],
                                     rhs=X[:, :din], start=(t == 0),
                                     stop=(t == NT - 1))
                    for a in range(nh):
                        ka = lastcols[a]
                        nc.tensor.matmul(
                            out=S_ps[a][:ka, :din], lhsT=X[:, a * P:a * P + ka],
                            rhs=X[:, :din], start=(t == 0), stop=(t == NT - 1))
                    # transpose X -> XT, spill to DRAM
                    xt_ps = pp.tile([P, HID], F32, tag="xt", bufs=2, name="xtps")
                    for a in range(nh):
                        ka = lastcols[a]
                        nc.tensor.transpose(
                            out=xt_ps[:ka, a * P:a * P + P],
                            in_=X[:, a * P:a * P + ka],
                            identity=ident_sb[:],
                        )
                    xt_sb = plxt.tile([P, HID], F32, tag="xtsb", name="xtsb")
                    for a in range(nh):
                        ka = lastcols[a]
                        nc.scalar.copy(out=xt_sb[:ka, a * P:a * P + P],
                                       in_=xt_ps[:ka, a * P:a * P + P])
                    kmax = min(P, din)
                    nc.scalar.dma_start(
                        out=xt_dram[:kmax, t * HID:t * HID + nh * P],
                        in_=xt_sb[:kmax, :nh * P])

                # ---------- per-layer stats & BN folding ----------
                S_sb = []
                for a in range(nh):
                    ka = lastcols[a]
                    st = plst.tile([P, HID], F32, tag=f"Ssb{a}", name=f"Ssb{a}")
                    nc.scalar.copy(out=st[:ka, :din], in_=S_ps[a][:ka, :din])
                    S_sb.append(st)
                sumX_sb = plst.tile([1, HID], F32, tag="sumXsb", name="sumXsb")
                nc.scalar.copy(out=sumX_sb[:, :din], in_=sumX_ps[0:1, :din])
                # sumY2 = colsum(W * (S @ W))   (S symmetric)
                y2s_ps = pp.tile([P, HID], F32, tag="agg", bufs=3, name="y2sps")
                for b in range(nh):
                    kb = lastcols[b]
                    SW_ps = pp.tile([P, HID], F32, tag="xt", bufs=2, name="SWps")
                    for a in range(nh):
                        ka = lastcols[a]
                        nc.tensor.matmul(
                            out=SW_ps[:kb, :],
                            lhsT=S_sb[a][:ka, b * P:b * P + kb],
                            rhs=W_sb[l][a][:ka, :],
                            start=(a == 0), stop=(a == nh - 1))
                    wsw = plst.tile([P, HID], F32, tag="wsw", name="wsw")
                    nc.vector.tensor_mul(out=wsw[:kb, :], in0=W_sb[l][b][:kb, :],
                                         in1=SW_ps[:kb, :])
                    nc.tensor.matmul(out=y2s_ps[0:1, :], lhsT=ones_sb[:kb, 0:1],
                                     rhs=wsw[:kb, :], start=(b == 0),
                                     stop=(b == nh - 1))
                # sumY = sumX @ W : transpose sumX, then matmul
                sxc = plst.tile([P, 2], F32, tag="sxc", name="sxc")
                for a in range(nh):
                    ka = lastcols[a]
                    tp = pp.tile([P, HID], F32, tag="xt", bufs=2, name="tp1")
                    nc.tensor.transpose(out=tp[:ka, 0:1],
                                        in_=sumX_sb[0:1, a * P:a * P + ka],
                                        identity=ident_sb[0:1, 0:1])
                    nc.scalar.copy(out=sxc[:ka, a:a + 1], in_=tp[:ka, 0:1])
                sy_ps = pp.tile([P, HID], F32, tag="xt", bufs=2, name="syps")
                for a in range(nh):
                    ka = lastcols[a]
                    nc.tensor.matmul(out=sy_ps[0:1, :], lhsT=sxc[:ka, a:a + 1],
                                     rhs=W_sb[l][a][:ka, :], start=(a == 0),
                                     stop=(a == nh - 1))
                arp = plst.tile([1, 2 * HID], F32, tag="arp", name="arp")
                nc.scalar.copy(out=arp[0:1, :HID], in_=sy_ps[0:1, :])
                nc.scalar.copy(out=arp[0:1, HID:], in_=y2s_ps[0:1, :])
                nc.sync.dma_start(out=ar_in[l][:], in_=arp[:])
                if coll:
                    nc.gpsimd.collective_compute(
                        "AllReduce", OP.add,
                        replica_groups=[list(range(NC_))],
                        ins=[ar_in[l][:]], outs=[ar_out[l][:]],
                    )
                else:
                    nc.sync.dma_start(out=ar_out[l][:], in_=ar_in[l][:])
                arr = plst.tile([1, 2 * HID], F32, tag="arr", name="arr")
                nc.sync.dma_start(out=arr[:], in_=ar_out[l][:])
                mean = plst.tile([1, HID], F32, tag="mean", name="mean")
                nc.scalar.mul(out=mean[:], in_=arr[0:1, :HID], mul=1.0 / N)
                ex2 = plst.tile([1, HID], F32, tag="stmp0", name="ex2")
                nc.scalar.mul(out=ex2[:], in_=arr[0:1, HID:], mul=1.0 / N)
                msq = plst.tile([1, HID], F32, tag="stmp1", name="msq")
                nc.scalar.activation(out=msq[:], in_=mean[:], func=AF.Square)
                var = plst.tile([1, HID], F32, tag="stmp2", name="var")
                nc.vector.tensor_sub(out=var[:], in0=ex2[:], in1=msq[:])
                vep = plst.tile([1, HID], F32, tag="stmp0", name="vep")
                nc.vector.tensor_scalar_add(out=vep[:], in0=var[:], scalar1=EPS_BN)
                rv = plst.tile([1, HID], F32, tag="stmp1", name="rv")
                nc.vector.reciprocal(out=rv[:], in_=vep[:])
                rs = plst.tile([1, HID], F32, tag="stmp2", name="rs")
                nc.scalar.activation(out=rs[:], in_=rv[:], func=AF.Sqrt)
                A_sb = plst.tile([1, HID], F32, tag="A_sb", name="A_sb")
                nc.vector.tensor_mul(out=A_sb[:], in0=rs[:], in1=bnp_sb[0:1, l * HID:(l + 1) * HID])
                mA = plst.tile([1, HID], F32, tag="stmp0", name="mA")
                nc.vector.tensor_mul(out=mA[:], in0=mean[:], in1=A_sb[:])
                B_sb = plst.tile([1, HID], F32, tag="B_sb", name="B_sb")
                nc.vector.tensor_sub(out=B_sb[:], in0=bnp_sb[0:1, (3 + l) * HID:(4 + l) * HID],
                                     in1=mA[:])
                arep_ps = pp.tile([P, HID], F32, tag="xt", bufs=2, name="arepps")
                nc.tensor.matmul(out=arep_ps[:], lhsT=ones_sb[0:1, :],
                                 rhs=A_sb[:], start=True, stop=True)
                Weff = []
                for a in range(nh):
                    ka = lastcols[a]
                    we = plweff.tile([P, HID], F32, tag="weff", name="weff")
                    nc.vector.tensor_mul(out=we[:ka, :], in0=W_sb[l][a][:ka, :],
                                         in1=arep_ps[:ka, :])
                    Weff.append(we)

                # ---------- pass 2 ----------
                if l == 2:
                    xread_ps = pp.tile([P, HID], F32, tag="Sps", bufs=3,
                                       name="xreadps")
                for t in range(NT):
                    nodes_t = min(P, SHARD - t * P)
                    kmax = min(P, din)
                    xt_ld = plxt.tile([P, HID], F32, tag="xtld", name="xtld")
                    nc.scalar.dma_start(out=xt_ld[:kmax, :nh * P],
                                        in_=xt_dram[:kmax, t * HID:t * HID + nh * P])
                    y2 = pp.tile([P, max(GQ, HID)], F32, tag="xt", bufs=2,
                                 name="y2")
                    for a in range(nh):
                        ka = lastcols[a]
                        nc.tensor.matmul(
                            out=y2[:, :HID],
                            lhsT=xt_ld[:ka, a * P:a * P + P],
                            rhs=Weff[a][:ka, :], start=(a == 0), stop=False)
                    nc.tensor.matmul(out=y2[:, :HID], lhsT=ones_sb[0:1, :],
                                     rhs=B_sb[:], start=False, stop=True)
                    hnew = plh.tile([P, HID], F32, tag="hnew", name="hnew")
                    nc.scalar.activation(out=hnew[:], in_=y2[:, :HID], func=AF.Relu)
                    if l < 2:
                        nc.sync.dma_start(
                            out=agin[t * P:t * P + nodes_t, :],
                            in_=hnew[:nodes_t, :])
                        qends = np.cumsum(TQ) - 1
                        if t in qends:
                            qq = int(np.where(qends == t)[0][0])
                            if coll:
                                nc.gpsimd.collective_compute(
                                    "AllGather", OP.bypass,
                                    replica_groups=[list(range(NC_))],
                                    ins=[agin[QOFF[qq]:QOFF[qq] + QB[qq], :]],
                                    outs=[h_blk[l][qq][:]],
                                )
                            else:
                                nc.sync.dma_start(
                                    out=h_blk[l][qq][0:QB[qq], :],
                                    in_=agin[QOFF[qq]:QOFF[qq] + QB[qq], :])
                    else:
                        R = plM.tile([P, P], F32, tag="R", name="R")
                        nc.vector.tensor_tensor(
                            out=R[:], in0=iota_sb[:],
                            in1=gidloc_sb[:, t:t + 1].to_broadcast([P, P]),
                            op=OP.is_equal)
                        nc.tensor.matmul(out=xread_ps[:], lhsT=R[:], rhs=hnew[:],
                                         start=(t == 0), stop=(t == NT - 1))

            # ---------------- readout exchange ----------------
            xp_sb = plst.tile([P, HID], F32, tag="xp_sb", name="xp_sb")
            nc.scalar.copy(out=xp_sb[:], in_=xread_ps[:])
            nc.sync.dma_start(out=xag_in[:], in_=xp_sb[:])
            if coll:
                nc.gpsimd.collective_compute(
                    "AllGather", OP.bypass,
                    replica_groups=[list(range(NC_))],
                    ins=[xag_in[:]], outs=[xag_out[:]],
                )
            else:
                nc.sync.dma_start(out=xag_out[0:P, :], in_=xag_in[:])
            xg_sb = plst.tile([P, NC_ * HID], F32, tag="xg_sb", name="xg_sb")
            nc.sync.dma_start(
                out=xg_sb[:].rearrange("p (c f) -> p c f", f=HID),
                in_=xag_out[:].rearrange("(c p) f -> p c f", p=P))
            xfull_sb = []
            for gt in range(GT):
                xf_ps = pp.tile([P, HID], F32, tag="agg", bufs=3, name="xfps")
                for c in range(NC_):
                    gsh = plM.tile([P, 1], F32, tag="gsh", name="gsh")
                    nc.vector.tensor_scalar(
                        out=gsh[:], in0=growid_sb[:, c:c + 1],
                        scalar1=float(P * gt), scalar2=None, op0=OP.subtract)
                    R2 = plM.tile([P, P], F32, tag="R", name="R2")
                    nc.vector.tensor_tensor(
                        out=R2[:], in0=iota_sb[:],
                        in1=gsh[:].to_broadcast([P, P]), op=OP.is_equal)
                    nc.tensor.matmul(out=xf_ps[:],
                                     lhsT=R2[:],
                                     rhs=xg_sb[:, c * HID:(c + 1) * HID],
                                     start=(c == 0), stop=(c == NC_ - 1))
                xf = plst.tile([P, HID], F32, tag=f"xfull{gt}", name=f"xfull{gt}")
                nc.scalar.copy(out=xf[:], in_=xf_ps[:])
                xfull_sb.append(xf)
            # transpose -> xT [HID(2 tiles), GQ]
            xT_sb = [plst.tile([P, GQ], F32, tag=f"xT{h}", name=f"xT{h}")
                     for h in range(HID // P)]
            for h in range(HID // P):
                for gt in range(GT):
                    tp2 = pp.tile([P, HID], F32, tag="xt", bufs=2, name="tp2")
                    nc.tensor.transpose(out=tp2[:, :P],
                                        in_=xfull_sb[gt][:, h * P:(h + 1) * P],
                                        identity=ident_sb[:])
                    nc.scalar.copy(out=xT_sb[h][:, gt * P:(gt + 1) * P],
                                   in_=tp2[:, :P])

            def bn_relu_T(y_ps, g_ap, b_ap, nb, tagsfx):
                """BN(train)+relu on feature-major psum tile [128, nb]."""
                s1 = plst.tile([P, 1], F32, tag="s1", name="s1")
                nc.vector.tensor_reduce(out=s1[:], in_=y_ps[:, :nb],
                                        axis=AX.X, op=OP.add)
                sq = plst.tile([P, GQ], F32, tag="sq", name="sq")
                s2 = plst.tile([P, 1], F32, tag="s2", name="s2")
                nc.scalar.activation(out=sq[:, :nb], in_=y_ps[:, :nb],
                                     func=AF.Square, accum_out=s2[:])
                mn = plst.tile([P, 1], F32, tag="mn", name="mn")
                nc.scalar.mul(out=mn[:], in_=s1[:], mul=1.0 / nb)
                e2 = plst.tile([P, 1], F32, tag="e2", name="e2")
                nc.scalar.mul(out=e2[:], in_=s2[:], mul=1.0 / nb)
                ms = plst.tile([P, 1], F32, tag="ms", name="ms")
                nc.scalar.activation(out=ms[:], in_=mn[:], func=AF.Square)
                vr = plst.tile([P, 1], F32, tag="vr", name="vr")
                nc.vector.tensor_sub(out=vr[:], in0=e2[:], in1=ms[:])
                ve = plst.tile([P, 1], F32, tag="ve", name="ve")
                nc.vector.tensor_scalar_add(out=ve[:], in0=vr[:], scalar1=EPS_BN)
                rv2 = plst.tile([P, 1], F32, tag="rv2", name="rv2")
                nc.vector.reciprocal(out=rv2[:], in_=ve[:])
                rs2 = plst.tile([P, 1], F32, tag="rs2", name="rs2")
                nc.scalar.activation(out=rs2[:], in_=rv2[:], func=AF.Sqrt)
                Am = plst.tile([P, 1], F32, tag="Am", name="Am")
                nc.vector.tensor_mul(out=Am[:], in0=rs2[:], in1=g_ap)
                mAm = plst.tile([P, 1], F32, tag="mAm", name="mAm")
                nc.vector.tensor_mul(out=mAm[:], in0=mn[:], in1=Am[:])
                Bm = plst.tile([P, 1], F32, tag="Bm", name="Bm")
                nc.vector.tensor_sub(out=Bm[:], in0=b_ap, in1=mAm[:])
                yo = plst.tile([P, GQ], F32, tag="yo", bufs=6, name=f"yo{tagsfx}")
                if nb < GQ:
                    nc.vector.memset(yo[:], 0.0)
                nc.scalar.activation(out=yo[:, :nb], in_=y_ps[:, :nb], func=AF.Relu,
                                     scale=Am[:], bias=Bm[:])
                return yo

            # FC1 -> BN -> relu (feature-major), then hidden MLP, then FC2
            y1t = []
            for m in range(cfg.MLP0 // P):
                y1_ps = pp.tile([P, max(GQ, HID)], F32, tag="xt", bufs=2,
                                name="y1ps")
                for k in range(HID // P):
                    nc.tensor.matmul(out=y1_ps[:, :GQ],
                                     lhsT=fc1_sb[k][:, m * P:(m + 1) * P],
                                     rhs=xT_sb[k][:], start=(k == 0),
                                     stop=(k == HID // P - 1))
                y1t.append(bn_relu_T(y1_ps, bn1g_sb[:, m:m + 1],
                                     bn1b_sb[:, m:m + 1], cfg.G, f"a{m}"))
            y2t = []
            for m in range(cfg.MLP1 // P):
                y2_ps = pp.tile([P, max(GQ, HID)], F32, tag="xt", bufs=2,
                                name="y2ps")
                for k in range(cfg.MLP0 // P):
                    nc.tensor.matmul(out=y2_ps[:, :GQ],
                                     lhsT=mlpW_sb[k][:, m * P:(m + 1) * P],
                                     rhs=y1t[k][:], start=(k == 0),
                                     stop=(k == cfg.MLP0 // P - 1))
                y2t.append(bn_relu_T(y2_ps, mbng_sb[:, m:m + 1],
                                     mbnb_sb[:, m:m + 1], cfg.G, f"b{m}"))
            for gt in range(GT):
                ng = min(P, cfg.G - gt * P)
                lg_ps = pp.tile([P, HID], F32, tag="agg", bufs=3, name="lgps")
                for k in range(cfg.MLP1 // P):
                    nc.tensor.matmul(out=lg_ps[:, :cfg.NCLS],
                                     lhsT=y2t[k][:, gt * P:gt * P + P],
                                     rhs=fc2_sb[k][:], start=(k == 0), stop=False)
                nc.tensor.matmul(out=lg_ps[:, :cfg.NCLS], lhsT=ones_sb[0:1, :],
                                 rhs=fc2b_sb[:], start=False, stop=True)
                ot = plh.tile([P, cfg.NCLS], F32, tag="ot", name="ot")
                nc.scalar.activation(out=ot[:ng, :], in_=lg_ps[:ng, :cfg.NCLS],
                                     func=AF.Sigmoid)
                nc.sync.dma_start(out=out_d[gt * P:gt * P + ng, :], in_=ot[:ng, :])


def _pack_inputs(inputs, cfg, meta):
    """Build per-core in_maps."""
    NC_ = cfg.NC
    h = np.ascontiguousarray(np.asarray(inputs["h"], np.float32))
    h0p = np.zeros((cfg.N, cfg.DPAD), np.float32)
    perm = np.empty(cfg.N, np.int64)
    pos = 0
    for b in range(cfg.NBLK):
        for c in range(NC_):
            s0 = c * cfg.SHARD + cfg.QOFF[b]
            perm[pos:pos + cfg.QB[b]] = np.arange(s0, s0 + cfg.QB[b])
            pos += cfg.QB[b]
    h0p[:, :cfg.IN_FEATS] = h[perm]
    iota = np.ascontiguousarray(
        np.tile(np.arange(P, dtype=np.float32)[None, :], (P, 1)))
    twoI = np.ascontiguousarray(2.0 * np.eye(P, dtype=np.float32))
    ident = np.ascontiguousarray(np.eye(P, dtype=np.float32))
    ones = np.ones((P, P), np.float32)
    Ws = [np.ascontiguousarray(np.asarray(w, np.float32)) for w in inputs["gcn_Ws"]]
    bnp = np.ascontiguousarray(np.concatenate(
        [np.asarray(x, np.float32) for x in inputs["bn_gs"]] +
        [np.asarray(x, np.float32) for x in inputs["bn_bs"]])[None, :])
    fc1W = np.ascontiguousarray(np.asarray(inputs["fc1_W"], np.float32))
    bn1g = np.ascontiguousarray(
        np.asarray(inputs["bn1_g"], np.float32).reshape(-1, P).T)
    bn1b = np.ascontiguousarray(
        np.asarray(inputs["bn1_b"], np.float32).reshape(-1, P).T)
    mlpW = np.ascontiguousarray(np.asarray(inputs["mlp_Ws"][0], np.float32))
    mbng = np.ascontiguousarray(
        np.asarray(inputs["mlp_bn_gs"][0], np.float32).reshape(-1, P).T)
    mbnb = np.ascontiguousarray(
        np.asarray(inputs["mlp_bn_bs"][0], np.float32).reshape(-1, P).T)
    fc2W = np.ascontiguousarray(
        np.asarray(inputs["fc2_W"], np.float32)[:, -cfg.NCLS:])
    fc2b = np.ascontiguousarray(
        np.asarray(inputs["fc2_b"], np.float32)[None, -cfg.NCLS:])
    in_maps = []
    for c in range(NC_):
        in_maps.append(dict(
            h0p=h0p,
            hself0=np.ascontiguousarray(h[c * cfg.SHARD:(c + 1) * cfg.SHARD]),
            idx16=np.ascontiguousarray(meta["idx16"][c]),
            dstloc=np.ascontiguousarray(meta["dstloc"][c]),
            gidloc=np.ascontiguousarray(meta["gidloc"][c]),
            growid=np.ascontiguousarray(meta["growid"]),
            iota=iota, twoI=twoI, ident=ident, ones=ones,
            W0=Ws[0], W1=Ws[1], W2=Ws[2], bnp=bnp,
            fc1W=fc1W, bn1gT=bn1g, bn1bT=bn1b,
            mlpW=mlpW, mbngT=mbng, mbnbT=mbnb,
            fc2Wr=fc2W, fc2br=fc2b,
        ))
    return in_maps


def make_nc(cfg, meta, coll=True, gathers=True, num_devices=None):
    nc = bacc.Bacc("TRN2", target_bir_lowering=False, debug=False,
                   enable_asserts=False,
                   num_devices=num_devices or (cfg.NC if coll else 1))
    _build(nc, cfg, meta, coll=coll, gathers=gathers)
    nc.compile()
    return nc


def build_and_run(inputs, cfg, **run_kwargs):
    meta = _plan(inputs["src"], inputs["dst"], inputs["graph_id"], cfg)
    nc = make_nc(cfg, meta)
    in_maps = _pack_inputs(inputs, cfg, meta)
    res = run_bass_kernel_spmd(nc, in_maps, core_ids=list(range(cfg.NC)),
                               **run_kwargs)
    return res


def kernel(**inputs):
    cfg = Cfg()
    res = build_and_run(inputs, cfg)
    return np.asarray(res.results[0]["out"], np.float32)


# revision 25
# speedup vs baseline: 1.1383x; 1.0324x over previous
"""GIN message-passing classifier on 8 Trainium2 NeuronCores.

Strategy (graph/node partition, data parallel):
  - Nodes are split into 8 equal contiguous shards (12500 nodes/core); each
    core owns the edges whose *destination* lands in its shard.
  - Host pre-sorts edges by dst, groups them per 128-node tile, pads each
    tile's edge list to a multiple of 128 ("chunks").  Chunk counts are taken
    as the max over cores so all 8 cores run one identical program (SPMD).
  - On device, chunks of 128 edges are gathered with batched indirect DMA
    (h[src] rows) and scatter-added into the tile's PSUM accumulator with a
    one-hot selector matmul built on the fly by the vector engine (is_equal
    against an iota row).  The GIN self term (2*h) is one extra matmul with a
    constant 2*I selector on contiguously-loaded own rows.
  - BatchNorm batch statistics come from per-core partials (sum(X) and the
    second moment X^T X pushed through W analytically), combined with a tiny
    [2,256] AllReduce; scale/shift are folded into W on device, so the
    per-node epilogue is matmul + fused relu.
  - h_new is exchanged between layers with an AllGather (rows = node shards).
  - Readout (segment-sum per graph) uses the same one-hot-selector matmul
    into per-core local graph slots, a small AllGather, and a host-planned
    slot->graph selector reduce; the graph-level MLP is replicated.
"""

import numpy as np

import concourse.bass as bass
import concourse.mybir as mybir
import concourse.tile as tile
from concourse import bacc
from concourse.bass_utils import run_bass_kernel_spmd

P = 128


def _T(tc, *args, **kw):
    t, _free = tc.tile(*args, **kw)
    return t


F32 = mybir.dt.float32
I32 = mybir.dt.int32
AF = mybir.ActivationFunctionType
OP = mybir.AluOpType
AX = mybir.AxisListType
EPS_BN = 1e-5


class Cfg:
    def __init__(self, N=100000, E=1000000, G=512, IN_FEATS=78, HID=256,
                 MLP0=512, MLP1=256, NCLS=204, NC=8, K_GATHER=8):
        self.N, self.E, self.G = N, E, G
        self.IN_FEATS, self.HID = IN_FEATS, HID
        self.MLP0, self.MLP1, self.NCLS = MLP0, MLP1, NCLS
        self.NC, self.K = NC, K_GATHER
        assert N % NC == 0
        self.SHARD = N // NC
        self.NT = (self.SHARD + P - 1) // P
        self.GT = (G + P - 1) // P
        self.NBLK = 4
        # block b = all cores' quarter-b of their shard; quarters are
        # tile-aligned so pipelined AllGathers unblock gather segments.
        tq = (self.NT + 3) // 4           # tiles per quarter (last smaller)
        self.TQ = [tq, tq, tq, self.NT - 3 * tq]
        qb = [min(t * P, self.SHARD) for t in np.cumsum([0] + self.TQ)]
        self.QOFF = qb[:4]                # row offset of quarter q in shard
        self.QB = [qb[i + 1] - qb[i] for i in range(4)]  # rows per quarter
        self.BSZ = [self.NC * q for q in self.QB]        # rows per block
        self.BLKOFF = [0]
        for b in range(3):
            self.BLKOFF.append(self.BLKOFF[-1] + self.BSZ[b])
        assert max(self.BSZ) < 32768, "dma_gather int16 index range"
        self.DPAD = 128  # layer-0 gather row padded to 128 f32 (512B)


def _plan(src, dst, graph_id, cfg):
    """Host-side edge bucketing. Returns per-core data + shared metadata.

    Edges are bucketed by (dst-tile, src-block); the chunk stream is laid out
    block-major (all tiles' block-0 chunks, then block-1, ...) so each
    dma_gather instruction covers one contiguous same-block run of chunks.
    """
    NC, SHARD, NT = cfg.NC, cfg.SHARD, cfg.NT
    NBLK = cfg.NBLK
    qoff = np.asarray(cfg.QOFF + [SHARD], np.int64)
    qb = np.asarray(cfg.QB, np.int64)
    src = np.asarray(src).astype(np.int64).ravel()
    dst = np.asarray(dst).astype(np.int64).ravel()
    gid = np.asarray(graph_id).astype(np.int64).ravel()

    core = dst // SHARD
    rem = dst % SHARD
    t = rem // P
    loc = rem % P
    score = src // SHARD
    soff = src % SHARD
    b = np.searchsorted(qoff, soff, side="right") - 1
    key = (core * NT + t) * NBLK + b
    srcl_all = score * qb[b] + (soff - qoff[b])
    # sort within each (core,tile,block) cell by source row: orders gather
    # descriptors by ascending HBM address within each instruction
    order = np.lexsort((srcl_all, key))
    key = key[order]
    srcl = srcl_all[order]
    assert srcl.max() < 32768
    loc = loc[order]
    cnt = np.bincount(key, minlength=NC * NT * NBLK).reshape(NC, NT, NBLK)
    c_tb = ((cnt + P - 1) // P).max(axis=0).astype(np.int64)  # [NT, NBLK]
    segcnt = c_tb.sum(axis=0)                                  # [NBLK]
    seg0 = np.zeros(NBLK, np.int64)
    seg0[1:] = np.cumsum(segcnt)[:-1]
    # chunk-stream start of each (t, b) run
    colb0 = np.zeros((NT, NBLK), np.int64)
    for bb in range(NBLK):
        colb0[0, bb] = seg0[bb]
        colb0[1:, bb] = seg0[bb] + np.cumsum(c_tb[:-1, bb])
    C = max(int(c_tb.sum()), 1)

    starts = np.zeros(NC * NT * NBLK + 1, np.int64)
    starts[1:] = np.cumsum(cnt.ravel())
    rank = np.arange(len(key)) - starts[key]
    ch = rank // P
    p = rank % P
    tt = (key // NBLK) % NT
    bb_ = key % NBLK
    cc = key // (NT * NBLK)
    s = colb0[tt, bb_] + ch  # stream chunk index
    g = s * P + p            # global slot position

    idx16 = np.zeros((NC, C * P), np.int16)
    dstloc = np.full((NC, P, C), -1.0, np.float32)
    idx16[cc, g] = srcl.astype(np.int16)
    dstloc[cc, p, s] = loc.astype(np.float32)
    # wrapped layout: position g -> [g % 16, g // 16], replicated on 8 Q7 cores
    idx16w = np.zeros((NC, P, (C * P) // 16), np.int16)
    for c in range(NC):
        w = idx16[c].reshape((C * P) // 16, 16).T  # [16, 8C]
        idx16w[c] = np.tile(w, (8, 1))

    gidloc = np.full((NC, P, NT), -1.0, np.float32)
    growid = np.full((P, NC), -1.0, np.float32)
    for c in range(NC):
        gg = gid[c * SHARD:(c + 1) * SHARD]
        gb, gm = int(gg[0]), int(gg[-1])
        assert gm - gb < P, "graph span exceeds 128 per core"
        arr = np.full(NT * P, -1.0, np.float32)
        arr[:SHARD] = (gg - gb).astype(np.float32)
        gidloc[c] = arr.reshape(NT, P).T
        jj = np.arange(P)
        sel = (gb + jj) <= gm
        growid[sel, c] = (gb + jj[sel]).astype(np.float32)

    return dict(c_tb=c_tb, C=C, colb0=colb0, seg0=seg0, segcnt=segcnt,
                idx16=idx16w, dstloc=dstloc, gidloc=gidloc, growid=growid)


def _build(nc, cfg, meta, coll=True, gathers=True):
    """Trace the full Bass/Tile program (shared by all 8 cores)."""
    NT, C, K = cfg.NT, meta["C"], cfg.K
    c_tb, colb0 = meta["c_tb"], meta["colb0"]
    seg0, segcnt = meta["seg0"], meta["segcnt"]
    NBLK = cfg.NBLK
    BSZ, BLKOFF, QOFF, QB, TQ = cfg.BSZ, cfg.BLKOFF, cfg.QOFF, cfg.QB, cfg.TQ
    HID = cfg.HID
    DIMS = [cfg.IN_FEATS, HID, HID]
    N, SHARD, NC_ = cfg.N, cfg.SHARD, cfg.NC
    GT = cfg.GT
    GQ = GT * P

    # ---------------- DRAM I/O ----------------
    h0p = nc.dram_tensor("h0p", [N, cfg.DPAD], F32, kind="ExternalInput").ap()
    hself0 = nc.dram_tensor("hself0", [SHARD, cfg.IN_FEATS], F32,
                            kind="ExternalInput").ap()
    idx_d = nc.dram_tensor("idx16", [P, (C * P) // 16], mybir.dt.int16,
                           kind="ExternalInput").ap()
    dstloc_d = nc.dram_tensor("dstloc", [P, C], F32, kind="ExternalInput").ap()
    gidloc_d = nc.dram_tensor("gidloc", [P, NT], F32, kind="ExternalInput").ap()
    growid_d = nc.dram_tensor("growid", [P, NC_], F32, kind="ExternalInput").ap()
    iota_d = nc.dram_tensor("iota", [P, P], F32, kind="ExternalInput").ap()
    twoI_d = nc.dram_tensor("twoI", [P, P], F32, kind="ExternalInput").ap()
    ident_d = nc.dram_tensor("ident", [P, P], F32, kind="ExternalInput").ap()
    ones_d = nc.dram_tensor("ones", [P, P], F32, kind="ExternalInput").ap()
    W_d = [nc.dram_tensor(f"W{i}", [DIMS[i], HID], F32, kind="ExternalInput").ap()
           for i in range(3)]
    bnp_d = nc.dram_tensor("bnp", [1, 6 * HID], F32,
                           kind="ExternalInput").ap()
    fc1_d = nc.dram_tensor("fc1W", [HID, cfg.MLP0], F32, kind="ExternalInput").ap()
    bn1g_d = nc.dram_tensor("bn1gT", [P, cfg.MLP0 // P], F32,
                            kind="ExternalInput").ap()
    bn1b_d = nc.dram_tensor("bn1bT", [P, cfg.MLP0 // P], F32,
                            kind="ExternalInput").ap()
    mlpW_d = nc.dram_tensor("mlpW", [cfg.MLP0, cfg.MLP1], F32,
                            kind="ExternalInput").ap()
    mbng_d = nc.dram_tensor("mbngT", [P, cfg.MLP1 // P], F32,
                            kind="ExternalInput").ap()
    mbnb_d = nc.dram_tensor("mbnbT", [P, cfg.MLP1 // P], F32,
                            kind="ExternalInput").ap()
    fc2_d = nc.dram_tensor("fc2Wr", [cfg.MLP1, cfg.NCLS], F32,
                           kind="ExternalInput").ap()
    fc2b_d = nc.dram_tensor("fc2br", [1, cfg.NCLS], F32, kind="ExternalInput").ap()
    out_d = nc.dram_tensor("out", [cfg.G, cfg.NCLS], F32, kind="ExternalOutput").ap()

    with tile.TileContext(nc) as tc, \
            tc.tile_pool(name="plD", bufs=1, space="DRAM") as plD, \
            tc.tile_pool(name="plC", bufs=1) as plC:
        def DT(name, shape, addr_space="Local"):
            return plD.tile(shape, F32, tag=name, name=name,
                            addr_space=addr_space)

        def CT(name, shape, dtype=F32, src_ap=None):
            t = plC.tile(shape, dtype, tag=name, name=name)
            if src_ap is not None:
                nc.sync.dma_start(out=t[:], in_=src_ap)
            return t

        # shared DRAM scratch: per layer, 4 block buffers (block b = all
        # cores' quarter-b rows, rank-major within the block)
        h_blk = [[DT(f"hx{i}b{b}", [BSZ[b], HID], addr_space="Shared")
                  for b in range(NBLK)] for i in range(2)]
        agin = DT("agin", [SHARD, HID])
        ar_in = [DT(f"arin{i}", [1, 2 * HID]) for i in range(3)]
        ar_out = [DT(f"arout{i}", [1, 2 * HID], addr_space="Shared")
                  for i in range(3)]
        xag_in = DT("xagin", [P, HID])
        xag_out = DT("xagout", [NC_ * P, HID], addr_space="Shared")
        xt_dram = DT("xt_dram", [P, NT * HID])

        # ---------------- persistent SBUF ----------------
        idx_sb = CT("idx_sb", [P, (C * P) // 16], mybir.dt.int16, idx_d)
        dstloc_sb = CT("dstloc_sb", [P, C], F32, dstloc_d)
        gidloc_sb = CT("gidloc_sb", [P, NT], F32, gidloc_d)
        growid_sb = CT("growid_sb", [P, NC_], F32, growid_d)
        iota_sb = CT("iota_sb", [P, P], F32, iota_d)
        twoI_sb = CT("twoI_sb", [P, P], F32, twoI_d)
        ident_sb = CT("ident_sb", [P, P], F32, ident_d)
        ones_sb = CT("ones_sb", [P, P], F32, ones_d)

        W_sb = []  # per layer: list of [128, HID] k-half tiles
        for l in range(3):
            din = DIMS[l]
            halves = []
            for a in range((din + P - 1) // P):
                ka = min(P, din - a * P)
                wt = CT(f"Wsb{l}_{a}", [P, HID])
                nc.sync.dma_start(out=wt[:ka, :], in_=W_d[l][a * P:a * P + ka, :])
                halves.append(wt)
            W_sb.append(halves)
        bnp_sb = CT("bnp_sb", [1, 6 * HID], F32, bnp_d)

        fc1_sb = [CT(f"fc1sb{a}", [P, cfg.MLP0]) for a in range(HID // P)]
        for a in range(HID // P):
            nc.sync.dma_start(out=fc1_sb[a][:], in_=fc1_d[a * P:(a + 1) * P, :])
        bn1g_sb = CT("bn1g_sb", [P, cfg.MLP0 // P], F32, bn1g_d)
        bn1b_sb = CT("bn1b_sb", [P, cfg.MLP0 // P], F32, bn1b_d)
        mlpW_sb = [CT(f"mlpWsb{a}", [P, cfg.MLP1]) for a in range(cfg.MLP0 // P)]
        for a in range(cfg.MLP0 // P):
            nc.sync.dma_start(out=mlpW_sb[a][:], in_=mlpW_d[a * P:(a + 1) * P, :])
        mbng_sb = CT("mbng_sb", [P, cfg.MLP1 // P], F32, mbng_d)
        mbnb_sb = CT("mbnb_sb", [P, cfg.MLP1 // P], F32, mbnb_d)
        fc2_sb = [CT(f"fc2sb{a}", [P, cfg.NCLS]) for a in range(cfg.MLP1 // P)]
        for a in range(cfg.MLP1 // P):
            nc.sync.dma_start(out=fc2_sb[a][:], in_=fc2_d[a * P:(a + 1) * P, :])
        fc2b_sb = CT("fc2b_sb", [1, cfg.NCLS], F32, fc2b_d)

        cmax = max(int(c_tb.sum(axis=1).max()), 1)

        with (
            tc.tile_pool(name="plG", bufs=7) as plG,
            tc.tile_pool(name="plgs", bufs=3) as plgs,
            tc.tile_pool(name="plM", bufs=2) as plM,
            tc.tile_pool(name="plX", bufs=3) as plX,
            tc.tile_pool(name="plh", bufs=3) as plh,
            tc.tile_pool(name="plxt", bufs=3) as plxt,
            tc.tile_pool(name="plst", bufs=1) as plst,
            tc.tile_pool(name="plweff", bufs=2) as plweff,
            tc.tile_pool(name="pp", bufs=1, space="PSUM") as pp,
        ):
            for l in range(3):
                din = DIMS[l]
                dpad = cfg.DPAD if l == 0 else HID  # gather row width
                nh = (din + P - 1) // P  # k-halves
                hselfsrc = hself0 if l == 0 else agin
                lastcols = [min(P, din - a * P) for a in range(nh)]

                # per-layer persistent psum accumulators
                S_ps = [pp.tile([P, HID], F32, tag="Sps", bufs=3,
                                name=f"Sps{l}_{a}") for a in range(nh)]
                sumX_ps = pp.tile([P, HID], F32, tag="Sps", bufs=3,
                                  name=f"sumXps{l}") if nh < 3 else None
                if sumX_ps is None:
                    raise AssertionError("din>256 unsupported")

                g_tiles = {}
                # stream chunk s -> (group key, slot)
                chunk_grp = {}
                for bb in range(NBLK):
                    for gi in range((int(segcnt[bb]) + K - 1) // K):
                        s0g = int(seg0[bb]) + gi * K
                        kk = min(K, int(seg0[bb]) + int(segcnt[bb]) - s0g)
                        for sl in range(kk):
                            chunk_grp[s0g + sl] = (bb, gi, s0g, kk, sl)

                def get_group(s, dpad=dpad, l=l, g_tiles=g_tiles,
                              chunk_grp=chunk_grp):
                    bb, gi, s0g, kk, sl = chunk_grp[s]
                    gkey = (bb, gi)
                    if gkey not in g_tiles:
                        gt_ = plG.tile([P, K * HID], F32, tag="G", name="Gt")
                        if not gathers:
                            g_tiles[gkey] = gt_
                            return g_tiles[gkey], sl
                        if l == 0:
                            src_ap = h0p[BLKOFF[bb]:BLKOFF[bb] + BSZ[bb], :]
                        else:
                            src_ap = h_blk[l - 1][bb][:]
                        nc.gpsimd.dma_gather(
                            out_ap=gt_[:, :kk * dpad]
                                .rearrange("p (k d) -> p k d", d=dpad),
                            in_ap=src_ap,
                            idxs_ap=idx_sb[:, 8 * s0g:8 * (s0g + kk)],
                            num_idxs=P * kk,
                            num_idxs_reg=P * kk,
                            elem_size=dpad,
                            single_packet=False,
                        )
                        g_tiles[gkey] = gt_
                    return g_tiles[gkey], sl

                # ---------- pass 1 ----------
                for t in range(NT):
                    ct = int(c_tb[t].sum())
                    nodes_t = min(P, SHARD - t * P)
                    agg = pp.tile([P, HID], F32, tag="agg", bufs=3, name="agg")
                    gs = plgs.tile([P, HID], F32, tag="gs", name="gs")
                    if nodes_t < P:
                        nc.vector.memset(gs[:, :din], 0.0)
                    nc.sync.dma_start(
                        out=gs[:nodes_t, :din],
                        in_=hselfsrc[t * P:t * P + nodes_t, :din])
                    if ct > 0:
                        M = plM.tile([P, cmax * P], F32, tag="M", name="M")
                        jj0 = 0
                        for bb in range(NBLK):
                            ctb = int(c_tb[t][bb])
                            if ctb == 0:
                                continue
                            cb0 = int(colb0[t][bb])
                            nc.vector.tensor_tensor(
                                out=M[:, jj0 * P:(jj0 + ctb) * P]
                                    .rearrange("p (c f) -> p c f", f=P),
                                in0=iota_sb[:].rearrange("p (o f) -> p o f", o=1)
                                    .to_broadcast([P, ctb, P]),
                                in1=dstloc_sb[:, cb0:cb0 + ctb]
                                    .rearrange("p (c o) -> p c o", o=1)
                                    .to_broadcast([P, ctb, P]),
                                op=OP.is_equal,
                            )
                            jj0 += ctb
                        jj = 0
                        for bb in range(NBLK):
                            for j in range(int(c_tb[t][bb])):
                                s = int(colb0[t][bb]) + j
                                gt_, sl = get_group(s)
                                nc.tensor.matmul(
                                    out=agg[:, :din],
                                    lhsT=M[:, jj * P:(jj + 1) * P],
                                    rhs=gt_[:, sl * dpad:sl * dpad + din],
                                    start=(jj == 0), stop=False,
                                )
                                jj += 1
                    nc.tensor.matmul(
                        out=agg[:, :din], lhsT=twoI_sb[:], rhs=gs[:, :din],
                        start=(ct == 0), stop=True,
                    )
                    X = plX.tile([P, HID], F32, tag="X", name="X")
                    nc.vector.tensor_copy(out=X[:, :din], in_=agg[:, :din])
                    # stats
                    nc.tensor.matmul(out=sumX_ps[0:1, :din], lhsT=ones_sb[:, 0:1],
                                     rhs=X[:, :din], start=(t == 0),
                                     stop=(t == NT - 1))
                    for a in range(nh):
                        ka = lastcols[a]
                        nc.tensor.matmul(
                            out=S_ps[a][:ka, :din], lhsT=X[:, a * P:a * P + ka],
                            rhs=X[:, :din], start=(t == 0), stop=(t == NT - 1))
                    # transpose X -> XT, spill to DRAM
                    xt_ps = pp.tile([P, HID], F32, tag="xt", bufs=2, name="xtps")
                    for a in range(nh):
                        ka = lastcols[a]
                        nc.tensor.transpose(
                            out=xt_ps[:ka, a * P:a * P + P],
                            in_=X[:, a * P:a * P + ka],
                            identity=ident_sb[:],
                        )
                    xt_sb = plxt.tile([P, HID], F32, tag="xtsb", name="xtsb")
                    for a in range(nh):
                        ka = lastcols[a]
                        nc.scalar.copy(out=xt_sb[:ka, a * P:a * P + P],
                                       in_=xt_ps[:ka, a * P:a * P + P])
                    kmax = min(P, din)
                    nc.scalar.dma_start(
                        out=xt_dram[:kmax, t * HID:t * HID + nh * P],
                        in_=xt_sb[:kmax, :nh * P])

                # ---------- per-layer stats & BN folding ----------
                S_sb = []
                for a in range(nh):
                    ka = lastcols[a]
                    st = plst.tile([P, HID], F32, tag=f"Ssb{a}", name=f"Ssb{a}")
                    nc.scalar.copy(out=st[:ka, :din], in_=S_ps[a][:ka, :din])
                    S_sb.append(st)
                sumX_sb = plst.tile([1, HID], F32, tag="sumXsb", name="sumXsb")
                nc.scalar.copy(out=sumX_sb[:, :din], in_=sumX_ps[0:1, :din])
                # sumY2 = colsum(W * (S @ W))   (S symmetric)
                y2s_ps = pp.tile([P, HID], F32, tag="agg", bufs=3, name="y2sps")
                for b in range(nh):
                    kb = lastcols[b]
                    SW_ps = pp.tile([P, HID], F32, tag="xt", bufs=2, name="SWps")
                    for a in range(nh):
                        ka = lastcols[a]
                        nc.tensor.matmul(
                            out=SW_ps[:kb, :],
                            lhsT=S_sb[a][:ka, b * P:b * P + kb],
                            rhs=W_sb[l][a][:ka, :],
                            start=(a == 0), stop=(a == nh - 1))
                    wsw = plst.tile([P, HID], F32, tag="wsw", name="wsw")
                    nc.vector.tensor_mul(out=wsw[:kb, :], in0=W_sb[l][b][:kb, :],
                                         in1=SW_ps[:kb, :])
                    nc.tensor.matmul(out=y2s_ps[0:1, :], lhsT=ones_sb[:kb, 0:1],
                                     rhs=wsw[:kb, :], start=(b == 0),
                                     stop=(b == nh - 1))
                # sumY = sumX @ W : transpose sumX, then matmul
                sxc = plst.tile([P, 2], F32, tag="sxc", name="sxc")
                for a in range(nh):
                    ka = lastcols[a]
                    tp = pp.tile([P, HID], F32, tag="xt", bufs=2, name="tp1")
                    nc.tensor.transpose(out=tp[:ka, 0:1],
                                        in_=sumX_sb[0:1, a * P:a * P + ka],
                                        identity=ident_sb[0:1, 0:1])
                    nc.scalar.copy(out=sxc[:ka, a:a + 1], in_=tp[:ka, 0:1])
                sy_ps = pp.tile([P, HID], F32, tag="xt", bufs=2, name="syps")
                for a in range(nh):
                    ka = lastcols[a]
                    nc.tensor.matmul(out=sy_ps[0:1, :], lhsT=sxc[:ka, a:a + 1],
                                     rhs=W_sb[l][a][:ka, :], start=(a == 0),
                                     stop=(a == nh - 1))
                arp = plst.tile([1, 2 * HID], F32, tag="arp", name="arp")
                nc.scalar.copy(out=arp[0:1, :HID], in_=sy_ps[0:1, :])
                nc.scalar.copy(out=arp[0:1, HID:], in_=y2s_ps[0:1, :])
                nc.sync.dma_start(out=ar_in[l][:], in_=arp[:])
                if coll:
                    nc.gpsimd.collective_compute(
                        "AllReduce", OP.add,
                        replica_groups=[list(range(NC_))],
                        ins=[ar_in[l][:]], outs=[ar_out[l][:]],
                    )
                else:
                    nc.sync.dma_start(out=ar_out[l][:], in_=ar_in[l][:])
                arr = plst.tile([1, 2 * HID], F32, tag="arr", name="arr")
                nc.sync.dma_start(out=arr[:], in_=ar_out[l][:])
                mean = plst.tile([1, HID], F32, tag="mean", name="mean")
                nc.scalar.mul(out=mean[:], in_=arr[0:1, :HID], mul=1.0 / N)
                ex2 = plst.tile([1, HID], F32, tag="stmp0", name="ex2")
                nc.scalar.mul(out=ex2[:], in_=arr[0:1, HID:], mul=1.0 / N)
                msq = plst.tile([1, HID], F32, tag="stmp1", name="msq")
                nc.scalar.activation(out=msq[:], in_=mean[:], func=AF.Square)
                var = plst.tile([1, HID], F32, tag="stmp2", name="var")
                nc.vector.tensor_sub(out=var[:], in0=ex2[:], in1=msq[:])
                vep = plst.tile([1, HID], F32, tag="stmp0", name="vep")
                nc.vector.tensor_scalar_add(out=vep[:], in0=var[:], scalar1=EPS_BN)
                rv = plst.tile([1, HID], F32, tag="stmp1", name="rv")
                nc.vector.reciprocal(out=rv[:], in_=vep[:])
                rs = plst.tile([1, HID], F32, tag="stmp2", name="rs")
                nc.scalar.activation(out=rs[:], in_=rv[:], func=AF.Sqrt)
                A_sb = plst.tile([1, HID], F32, tag="A_sb", name="A_sb")
                nc.vector.tensor_mul(out=A_sb[:], in0=rs[:], in1=bnp_sb[0:1, l * HID:(l + 1) * HID])
                mA = plst.tile([1, HID], F32, tag="stmp0", name="mA")
                nc.vector.tensor_mul(out=mA[:], in0=mean[:], in1=A_sb[:])
                B_sb = plst.tile([1, HID], F32, tag="B_sb", name="B_sb")
                nc.vector.tensor_sub(out=B_sb[:], in0=bnp_sb[0:1, (3 + l) * HID:(4 + l) * HID],
                                     in1=mA[:])
                arep_ps = pp.tile([P, HID], F32, tag="xt", bufs=2, name="arepps")
                nc.tensor.matmul(out=arep_ps[:], lhsT=ones_sb[0:1, :],
                                 rhs=A_sb[:], start=True, stop=True)
                Weff = []
                for a in range(nh):
                    ka = lastcols[a]
                    we = plweff.tile([P, HID], F32, tag="weff", name="weff")
                    nc.vector.tensor_mul(out=we[:ka, :], in0=W_sb[l][a][:ka, :],
                                         in1=arep_ps[:ka, :])
                    Weff.append(we)

                # ---------- pass 2 ----------
                if l == 2:
                    xread_ps = pp.tile([P, HID], F32, tag="Sps", bufs=3,
                                       name="xreadps")
                for t in range(NT):
                    nodes_t = min(P, SHARD - t * P)
                    kmax = min(P, din)
                    xt_ld = plxt.tile([P, HID], F32, tag="xtld", name="xtld")
                    nc.scalar.dma_start(out=xt_ld[:kmax, :nh * P],
                                        in_=xt_dram[:kmax, t * HID:t * HID + nh * P])
                    y2 = pp.tile([P, max(GQ, HID)], F32, tag="xt", bufs=2,
                                 name="y2")
                    for a in range(nh):
                        ka = lastcols[a]
                        nc.tensor.matmul(
                            out=y2[:, :HID],
                            lhsT=xt_ld[:ka, a * P:a * P + P],
                            rhs=Weff[a][:ka, :], start=(a == 0), stop=False)
                    nc.tensor.matmul(out=y2[:, :HID], lhsT=ones_sb[0:1, :],
                                     rhs=B_sb[:], start=False, stop=True)
                    hnew = plh.tile([P, HID], F32, tag="hnew", name="hnew")
                    nc.scalar.activation(out=hnew[:], in_=y2[:, :HID], func=AF.Relu)
                    if l < 2:
                        nc.sync.dma_start(
                            out=agin[t * P:t * P + nodes_t, :],
                            in_=hnew[:nodes_t, :])
                        qends = np.cumsum(TQ) - 1
                        if t in qends:
                            qq = int(np.where(qends == t)[0][0])
                            if coll:
                                nc.gpsimd.collective_compute(
                                    "AllGather", OP.bypass,
                                    replica_groups=[list(range(NC_))],
                                    ins=[agin[QOFF[qq]:QOFF[qq] + QB[qq], :]],
                                    outs=[h_blk[l][qq][:]],
                                )
                            else:
                                nc.sync.dma_start(
                                    out=h_blk[l][qq][0:QB[qq], :],
                                    in_=agin[QOFF[qq]:QOFF[qq] + QB[qq], :])
                    else:
                        R = plM.tile([P, P], F32, tag="R", name="R")
                        nc.vector.tensor_tensor(
                            out=R[:], in0=iota_sb[:],
                            in1=gidloc_sb[:, t:t + 1].to_broadcast([P, P]),
                            op=OP.is_equal)
                        nc.tensor.matmul(out=xread_ps[:], lhsT=R[:], rhs=hnew[:],
                                         start=(t == 0), stop=(t == NT - 1))

            # ---------------- readout exchange ----------------
            xp_sb = plst.tile([P, HID], F32, tag="xp_sb", name="xp_sb")
            nc.scalar.copy(out=xp_sb[:], in_=xread_ps[:])
            nc.sync.dma_start(out=xag_in[:], in_=xp_sb[:])
            if coll:
                nc.gpsimd.collective_compute(
                    "AllGather", OP.bypass,
                    replica_groups=[list(range(NC_))],
                    ins=[xag_in[:]], outs=[xag_out[:]],
                )
            else:
                nc.sync.dma_start(out=xag_out[0:P, :], in_=xag_in[:])
            xg_sb = plst.tile([P, NC_ * HID], F32, tag="xg_sb", name="xg_sb")
            nc.sync.dma_start(
                out=xg_sb[:].rearrange("p (c f) -> p c f", f=HID),
                in_=xag_out[:].rearrange("(c p) f -> p c f", p=P))
            xfull_sb = []
            for gt in range(GT):
                xf_ps = pp.tile([P, HID], F32, tag="agg", bufs=3, name="xfps")
                for c in range(NC_):
                    gsh = plM.tile([P, 1], F32, tag="gsh", name="gsh")
                    nc.vector.tensor_scalar(
                        out=gsh[:], in0=growid_sb[:, c:c + 1],
                        scalar1=float(P * gt), scalar2=None, op0=OP.subtract)
                    R2 = plM.tile([P, P], F32, tag="R", name="R2")
                    nc.vector.tensor_tensor(
                        out=R2[:], in0=iota_sb[:],
                        in1=gsh[:].to_broadcast([P, P]), op=OP.is_equal)
                    nc.tensor.matmul(out=xf_ps[:],
                                     lhsT=R2[:],
                                     rhs=xg_sb[:, c * HID:(c + 1) * HID],
                                     start=(c == 0), stop=(c == NC_ - 1))
                xf = plst.tile([P, HID], F32, tag=f"xfull{gt}", name=f"xfull{gt}")
                nc.scalar.copy(out=xf[:], in_=xf_ps[:])
                xfull_sb.append(xf)
            # transpose -> xT [HID(2 tiles), GQ]
            xT_sb = [plst.tile([P, GQ], F32, tag=f"xT{h}", name=f"xT{h}")
                     for h in range(HID // P)]
            for h in range(HID // P):
                for gt in range(GT):
                    tp2 = pp.tile([P, HID], F32, tag="xt", bufs=2, name="tp2")
                    nc.tensor.transpose(out=tp2[:, :P],
                                        in_=xfull_sb[gt][:, h * P:(h + 1) * P],
                                        identity=ident_sb[:])
                    nc.scalar.copy(out=xT_sb[h][:, gt * P:(gt + 1) * P],
                                   in_=tp2[:, :P])

            def bn_relu_T(y_ps, g_ap, b_ap, nb, tagsfx):
                """BN(train)+relu on feature-major psum tile [128, nb]."""
                s1 = plst.tile([P, 1], F32, tag="s1", name="s1")
                nc.vector.tensor_reduce(out=s1[:], in_=y_ps[:, :nb],
                                        axis=AX.X, op=OP.add)
                sq = plst.tile([P, GQ], F32, tag="sq", name="sq")
                s2 = plst.tile([P, 1], F32, tag="s2", name="s2")
                nc.scalar.activation(out=sq[:, :nb], in_=y_ps[:, :nb],
                                     func=AF.Square, accum_out=s2[:])
                mn = plst.tile([P, 1], F32, tag="mn", name="mn")
                nc.scalar.mul(out=mn[:], in_=s1[:], mul=1.0 / nb)
                e2 = plst.tile([P, 1], F32, tag="e2", name="e2")
                nc.scalar.mul(out=e2[:], in_=s2[:], mul=1.0 / nb)
                ms = plst.tile([P, 1], F32, tag="ms", name="ms")
                nc.scalar.activation(out=ms[:], in_=mn[:], func=AF.Square)
                vr = plst.tile([P, 1], F32, tag="vr", name="vr")
                nc.vector.tensor_sub(out=vr[:], in0=e2[:], in1=ms[:])
                ve = plst.tile([P, 1], F32, tag="ve", name="ve")
                nc.vector.tensor_scalar_add(out=ve[:], in0=vr[:], scalar1=EPS_BN)
                rv2 = plst.tile([P, 1], F32, tag="rv2", name="rv2")
                nc.vector.reciprocal(out=rv2[:], in_=ve[:])
                rs2 = plst.tile([P, 1], F32, tag="rs2", name="rs2")
                nc.scalar.activation(out=rs2[:], in_=rv2[:], func=AF.Sqrt)
                Am = plst.tile([P, 1], F32, tag="Am", name="Am")
                nc.vector.tensor_mul(out=Am[:], in0=rs2[:], in1=g_ap)
                mAm = plst.tile([P, 1], F32, tag="mAm", name="mAm")
                nc.vector.tensor_mul(out=mAm[:], in0=mn[:], in1=Am[:])
                Bm = plst.tile([P, 1], F32, tag="Bm", name="Bm")
                nc.vector.tensor_sub(out=Bm[:], in0=b_ap, in1=mAm[:])
                yo = plst.tile([P, GQ], F32, tag="yo", bufs=6, name=f"yo{tagsfx}")
                if nb < GQ:
                    nc.vector.memset(yo[:], 0.0)
                nc.scalar.activation(out=yo[:, :nb], in_=y_ps[:, :nb], func=AF.Relu,
                                     scale=Am[:], bias=Bm[:])
                return yo

            # FC1 -> BN -> relu (feature-major), then hidden MLP, then FC2
            y1t = []
            for m in range(cfg.MLP0 // P):
                y1_ps = pp.tile([P, max(GQ, HID)], F32, tag="xt", bufs=2,
                                name="y1ps")
                for k in range(HID // P):
                    nc.tensor.matmul(out=y1_ps[:, :GQ],
                                     lhsT=fc1_sb[k][:, m * P:(m + 1) * P],
                                     rhs=xT_sb[k][:], start=(k == 0),
                                     stop=(k == HID // P - 1))
                y1t.append(bn_relu_T(y1_ps, bn1g_sb[:, m:m + 1],
                                     bn1b_sb[:, m:m + 1], cfg.G, f"a{m}"))
            y2t = []
            for m in range(cfg.MLP1 // P):
                y2_ps = pp.tile([P, max(GQ, HID)], F32, tag="xt", bufs=2,
                                name="y2ps")
                for k in range(cfg.MLP0 // P):
                    nc.tensor.matmul(out=y2_ps[:, :GQ],
                                     lhsT=mlpW_sb[k][:, m * P:(m + 1) * P],
                                     rhs=y1t[k][:], start=(k == 0),
                                     stop=(k == cfg.MLP0 // P - 1))
                y2t.append(bn_relu_T(y2_ps, mbng_sb[:, m:m + 1],
                                     mbnb_sb[:, m:m + 1], cfg.G, f"b{m}"))
            for gt in range(GT):
                ng = min(P, cfg.G - gt * P)
                lg_ps = pp.tile([P, HID], F32, tag="agg", bufs=3, name="lgps")
                for k in range(cfg.MLP1 // P):
                    nc.tensor.matmul(out=lg_ps[:, :cfg.NCLS],
                                     lhsT=y2t[k][:, gt * P:gt * P + P],
                                     rhs=fc2_sb[k][:], start=(k == 0), stop=False)
                nc.tensor.matmul(out=lg_ps[:, :cfg.NCLS], lhsT=ones_sb[0:1, :],
                                 rhs=fc2b_sb[:], start=False, stop=True)
                ot = plh.tile([P, cfg.NCLS], F32, tag="ot", name="ot")
                nc.scalar.activation(out=ot[:ng, :], in_=lg_ps[:ng, :cfg.NCLS],
                                     func=AF.Sigmoid)
                nc.sync.dma_start(out=out_d[gt * P:gt * P + ng, :], in_=ot[:ng, :])


def _pack_inputs(inputs, cfg, meta):
    """Build per-core in_maps."""
    NC_ = cfg.NC
    h = np.ascontiguousarray(np.asarray(inputs["h"], np.float32))
    h0p = np.zeros((cfg.N, cfg.DPAD), np.float32)
    perm = np.empty(cfg.N, np.int64)
    pos = 0
    for b in range(cfg.NBLK):
        for c in range(NC_):
            s0 = c * cfg.SHARD + cfg.QOFF[b]
            perm[pos:pos + cfg.QB[b]] = np.arange(s0, s0 + cfg.QB[b])
            pos += cfg.QB[b]
    h0p[:, :cfg.IN_FEATS] = h[perm]
    iota = np.ascontiguousarray(
        np.tile(np.arange(P, dtype=np.float32)[None, :], (P, 1)))
    twoI = np.ascontiguousarray(2.0 * np.eye(P, dtype=np.float32))
    ident = np.ascontiguousarray(np.eye(P, dtype=np.float32))
    ones = np.ones((P, P), np.float32)
    Ws = [np.ascontiguousarray(np.asarray(w, np.float32)) for w in inputs["gcn_Ws"]]
    bnp = np.ascontiguousarray(np.concatenate(
        [np.asarray(x, np.float32) for x in inputs["bn_gs"]] +
        [np.asarray(x, np.float32) for x in inputs["bn_bs"]])[None, :])
    fc1W = np.ascontiguousarray(np.asarray(inputs["fc1_W"], np.float32))
    bn1g = np.ascontiguousarray(
        np.asarray(inputs["bn1_g"], np.float32).reshape(-1, P).T)
    bn1b = np.ascontiguousarray(
        np.asarray(inputs["bn1_b"], np.float32).reshape(-1, P).T)
    mlpW = np.ascontiguousarray(np.asarray(inputs["mlp_Ws"][0], np.float32))
    mbng = np.ascontiguousarray(
        np.asarray(inputs["mlp_bn_gs"][0], np.float32).reshape(-1, P).T)
    mbnb = np.ascontiguousarray(
        np.asarray(inputs["mlp_bn_bs"][0], np.float32).reshape(-1, P).T)
    fc2W = np.ascontiguousarray(
        np.asarray(inputs["fc2_W"], np.float32)[:, -cfg.NCLS:])
    fc2b = np.ascontiguousarray(
        np.asarray(inputs["fc2_b"], np.float32)[None, -cfg.NCLS:])
    in_maps = []
    for c in range(NC_):
        in_maps.append(dict(
            h0p=h0p,
            hself0=np.ascontiguousarray(h[c * cfg.SHARD:(c + 1) * cfg.SHARD]),
            idx16=np.ascontiguousarray(meta["idx16"][c]),
            dstloc=np.ascontiguousarray(meta["dstloc"][c]),
            gidloc=np.ascontiguousarray(meta["gidloc"][c]),
            growid=np.ascontiguousarray(meta["growid"]),
            iota=iota, twoI=twoI, ident=ident, ones=ones,
            W0=Ws[0], W1=Ws[1], W2=Ws[2], bnp=bnp,
            fc1W=fc1W, bn1gT=bn1g, bn1bT=bn1b,
            mlpW=mlpW, mbngT=mbng, mbnbT=mbnb,
            fc2Wr=fc2W, fc2br=fc2b,
        ))
    return in_maps


def make_nc(cfg, meta, coll=True, gathers=True, num_devices=None):
    nc = bacc.Bacc("TRN2", target_bir_lowering=False, debug=False,
                   enable_asserts=False,
                   num_devices=num_devices or (cfg.NC if coll else 1))
    _build(nc, cfg, meta, coll=coll, gathers=gathers)
    nc.compile()
    return nc


def build_and_run(inputs, cfg, **run_kwargs):
    meta = _plan(inputs["src"], inputs["dst"], inputs["graph_id"], cfg)
    nc = make_nc(cfg, meta)
    in_maps = _pack_inputs(inputs, cfg, meta)
    res = run_bass_kernel_spmd(nc, in_maps, core_ids=list(range(cfg.NC)),
                               **run_kwargs)
    return res


def kernel(**inputs):
    cfg = Cfg()
    res = build_and_run(inputs, cfg)
    return np.asarray(res.results[0]["out"], np.float32)
